# revision 1
# baseline (speedup 1.0000x reference)
"""Trainium2 Bass kernel for BasicSelfAttention (B=2, N=2048, C=1024, H=16, Dh=64).

Sharding: 8 cores = 2 batches x 4 head-groups. Core c handles batch c//4 and
heads [4*(c%4), 4*(c%4)+4). Each core:
  - qkv projection for its 768 output channels (natural [n, j] layout, fp32r)
  - LayerNorm(q), LayerNorm(k) over head_dim via free-dim reduces + broadcast APs
  - PE-transpose q,k to [d, n] layout; v stays natural [n, d] (+ ones column)
  - attention in transposed layout: S_t[j, i] = k_ln^T q_ln, exp without
    max-subtraction (logits bounded by LN: |s*scale| <= 8); causal handled by
    skipping fully-masked j-blocks, shrinking partial blocks to their live
    column range, and affine_select zeroing of the remaining triangle
  - PV matmul with ones-augmented V gives unnormalized O^T and the softmax
    denominator in one pass; normalize via reciprocal + DRAM-bounce broadcast
  - row-parallel output projection -> partial [2048, 1024]
Engine placement balances DVE (reduces, LN, normalize) against ACT (exp,
square, psum->sbuf copies) and Pool (v_aug copies, causal selects).
Host: pre-transposes x and weight slices, sums the 4 partial projections per
batch, adds b_proj.
"""

import numpy as np
from contextlib import ExitStack

import concourse.bass as bass
import concourse.mybir as mybir
import concourse.tile as tile
from concourse import bacc
from concourse.bass_utils import run_bass_kernel_spmd
from concourse.masks import make_identity

B, N, C, H, Dh = 2, 2048, 1024, 16, 64
HPC = 4                      # heads per core
NCORES = 8
SCALE = 8.0 / Dh             # 0.125 (use_mup)
EPS = 1e-5

F32 = mybir.dt.float32
F32R = mybir.dt.float32r
AF = mybir.ActivationFunctionType
OP = mybir.AluOpType

NB = N // 128                # 16 row blocks of 128
CB = C // 128                # 8 contraction blocks
IB = N // 512                # 4 query blocks of 512

_BUILD_CACHE = {}
LAST_RESULT = None


def _r(ap):
    return ap.bitcast(F32R)


def _bc3(ap2d, inner):
    """[p, g] AP -> [p, g, inner] with stride-0 inner dim."""
    return bass.AP(tensor=ap2d.tensor, offset=ap2d.offset,
                   ap=list(ap2d.ap) + [[0, inner]])


def _build(causal: bool, fast_gb: bool, exp_bias: float):
    nc = bacc.Bacc("TRN2", target_bir_lowering=False, debug=False,
                   num_devices=NCORES)

    xt_e = nc.dram_tensor("xt", [C, N], F32R, kind="ExternalInput")
    wqkv_e = nc.dram_tensor("wqkv_t", [C, 3 * HPC * Dh], F32R, kind="ExternalInput")
    wp_e = nc.dram_tensor("wp_t", [HPC * Dh, C], F32R, kind="ExternalInput")
    if not fast_gb:
        gt_e = nc.dram_tensor("g_bcast", [128, 512], F32, kind="ExternalInput")
        bt_e = nc.dram_tensor("b_bcast", [128, 512], F32, kind="ExternalInput")
    out_e = nc.dram_tensor("out_p", [N, C], F32, kind="ExternalOutput")

    with tile.TileContext(nc) as tc, ExitStack() as ctx:
        # ---- long-lived tiles ----
        persist = ctx.enter_context(tc.tile_pool(name="persist", bufs=1))
        ident_f = persist.tile([128, 128], F32, tag="identf")
        make_identity(nc, ident_f[:])
        ident = persist.tile([128, 128], F32R, tag="ident")
        nc.vector.tensor_copy(ident[:], ident_f[:])
        ones_t = persist.tile([128, 1], F32, tag="ones")
        nc.vector.memset(ones_t[:], 1.0)
        eps_t = persist.tile([128, 1], F32, tag="eps")
        nc.vector.memset(eps_t[:], EPS)
        eb_t = persist.tile([128, 1], F32, tag="ebias")
        nc.vector.memset(eb_t[:], exp_bias)

        wp_t = [persist.tile([128, C], F32R, tag=f"wp{p}", name=f"wp{p}")
                for p in range(2)]

        # transposed q|k, segments: 0,1 = q pairs; 2,3 = k pairs
        qkT = persist.tile([128, 4, N], F32R, tag="qkT")
        qT = [qkT[:, p, :] for p in range(2)]
        kT = [qkT[:, 2 + p, :] for p in range(2)]
        # OT (normalized attention output^T) per head-pair, input to proj
        oT = [persist.tile([128, N], F32R, tag=f"oT{p}", name=f"oTp{p}")
              for p in range(2)]

        if not fast_gb:
            gt = persist.tile([128, 512], F32, tag="gt")
            bt = persist.tile([128, 512], F32, tag="bt")
            nc.sync.dma_start(gt[:], gt_e[:])
            nc.sync.dma_start(bt[:], bt_e[:])

        va_pool = ctx.enter_context(tc.tile_pool(name="va", bufs=NB))
        va = [None] * NB

        # ---- phase 1: qkv projection + LN + transposes + v_aug ----
        with tc.tile_pool(name="ph1", bufs=1) as ph1, \
             tc.tile_pool(name="w1", bufs=3) as w1, \
             tc.tile_pool(name="ps1", bufs=2, space="PSUM") as ps1:
            xt = [ph1.tile([128, N], F32R, tag=f"xt{cb}", name=f"xt{cb}")
                  for cb in range(CB)]
            wq = [ph1.tile([128, 768], F32R, tag=f"wq{cb}", name=f"wq{cb}")
                  for cb in range(CB)]
            # wq fully + the first 512-column wave of xt arrive first, so
            # row-blocks 0-3 can finish their contraction after ~5MB of loads
            for cb in range(CB):
                nc.sync.dma_start(wq[cb][:], wqkv_e[128 * cb:128 * (cb + 1), :])
                nc.sync.dma_start(xt[cb][:, 0:256],
                                  xt_e[128 * cb:128 * (cb + 1), 0:256])
            for w0 in range(256, N, 256):
                for cb in range(CB):
                    nc.sync.dma_start(xt[cb][:, w0:w0 + 256],
                                      xt_e[128 * cb:128 * (cb + 1), w0:w0 + 256])
            # proj weights aren't needed until the projection ~100us in
            for p in range(2):
                nc.sync.dma_start(wp_t[p][:], wp_e[128 * p:128 * (p + 1), :])

            for nb in range(NB):
                n0 = 128 * nb
                qps = ps1.tile([128, 1024], F32, tag="qkvps", bufs=3)
                for cb in range(CB):
                    st = (cb == 0)
                    sp = (cb == CB - 1)
                    nc.tensor.matmul(qps[:, 0:512], _r(xt[cb][:, n0:n0 + 128]),
                                     _r(wq[cb][:, 0:512]), start=st, stop=sp)
                    nc.tensor.matmul(qps[:, 512:768], _r(xt[cb][:, n0:n0 + 128]),
                                     _r(wq[cb][:, 512:768]), start=st, stop=sp)
                nat = w1.tile([128, 768], F32R, tag="nat")
                nc.scalar.activation(nat[:], qps[:, 0:768], func=AF.Copy)

                # LayerNorm over each 64-wide group of q|k (8 groups)
                qk3 = nat[:, 0:512].rearrange("p (g d) -> p g d", g=8)
                sq = w1.tile([128, 512], F32, tag="sq")
                nc.scalar.activation(sq[:], nat[:, 0:512], func=AF.Square)
                sums = w1.tile([128, 8], F32, tag="sums")
                sqs = w1.tile([128, 8], F32, tag="sqs")
                nc.vector.tensor_reduce(sums[:], qk3, axis=mybir.AxisListType.X,
                                        op=OP.add)
                nc.vector.tensor_reduce(sqs[:],
                                        sq[:].rearrange("p (g d) -> p g d", g=8),
                                        axis=mybir.AxisListType.X, op=OP.add)
                mean = w1.tile([128, 8], F32, tag="mean")
                nc.vector.tensor_scalar(mean[:], sums[:], 1.0 / Dh, None,
                                        op0=OP.mult)
                msq = w1.tile([128, 8], F32, tag="msq")
                nc.vector.tensor_tensor(msq[:], mean[:], mean[:], op=OP.mult)
                rstd = w1.tile([128, 8], F32, tag="rstd")
                nc.vector.scalar_tensor_tensor(rstd[:], sqs[:], 1.0 / Dh, msq[:],
                                               op0=OP.mult, op1=OP.subtract)
                nc.scalar.activation(rstd[:], rstd[:], func=AF.Sqrt,
                                     bias=eps_t[:])
                nc.vector.reciprocal(rstd[:], rstd[:])
                # q_ln = (q - mean) * rstd, batched via stride-0 inner dims
                nc.vector.tensor_tensor(qk3, qk3, _bc3(mean[:], Dh),
                                        op=OP.subtract)
                nc.vector.tensor_tensor(qk3, qk3, _bc3(rstd[:], Dh), op=OP.mult)
                if not fast_gb:
                    nc.vector.tensor_tensor(nat[:, 0:512], nat[:, 0:512], gt[:],
                                            op=OP.mult)
                    nc.vector.tensor_tensor(nat[:, 0:512], nat[:, 0:512], bt[:],
                                            op=OP.add)

                # transpose q,k head-pairs; seg order [q0, q1, k0, k1]
                tr = ps1.tile([128, 512], F32R, tag="trps", bufs=1)
                for s in range(4):
                    nc.tensor.transpose(tr[:, 128 * s:128 * (s + 1)],
                                        nat[:, 128 * s:128 * (s + 1)], ident[:])
                nc.scalar.activation(qkT[:, :, n0:n0 + 128],
                                     tr[:].rearrange("p (s n) -> p s n", s=4),
                                     func=AF.Copy)

                # v_aug [128, 4, 65]
                vat = va_pool.tile([128, HPC, Dh + 1], F32R, tag="vat",
                                   name=f"vat{nb}")
                nc.gpsimd.tensor_copy(
                    vat[:, :, 0:Dh],
                    nat[:, 512:768].rearrange("p (h d) -> p h d", h=HPC))
                nc.gpsimd.tensor_copy(vat[:, :, Dh:Dh + 1],
                                      ones_t[:].to_broadcast([128, HPC, 1]))
                va[nb] = vat

        # ---- phase 2: attention ----
        with tc.tile_pool(name="pt", bufs=4) as ptp, \
             tc.tile_pool(name="rr", bufs=3) as rrp, \
             tc.tile_pool(name="rd", bufs=4, space="DRAM") as rdp, \
             tc.tile_pool(name="ps_s", bufs=3, space="PSUM") as ps_s, \
             tc.tile_pool(name="ps_o", bufs=2, space="PSUM") as ps_o:
            for ib in range(IB):
                for h in range(HPC):
                    p, off = h // 2, 64 * (h % 2)
                    i0 = 512 * ib
                    jmax = 4 * (ib + 1) if causal else NB

                    def width(jb):
                        if not causal or jb < 4 * ib:
                            return 512
                        return 512 - 128 * (jb - 4 * ib)

                    o_ps = ps_o.tile([Dh + 1, 512], F32, tag="ops",
                                     name=f"o{h}_{ib}")
                    for jp in range(jmax // 2):
                        jbs = (2 * jp, 2 * jp + 1)
                        ws = [width(jb) for jb in jbs]
                        s_ps = ps_s.tile([128, 1024], F32, tag="sps",
                                         name=f"s{h}_{ib}_{jp}")
                        for half, jb in enumerate(jbs):
                            w = ws[half]
                            # S block lives at columns [j0-i0, 512) of the
                            # i-range; store at [512*half, 512*half + w)
                            nc.tensor.matmul(
                                s_ps[:, 512 * half:512 * half + w],
                                _r(kT[p][off:off + Dh, 128 * jb:128 * (jb + 1)]),
                                _r(qT[p][off:off + Dh,
                                         i0 + 512 - w:i0 + 512]),
                                start=True, stop=True)
                        pt = ptp.tile([128, 1024], F32R, tag="pt",
                                      name=f"pt{h}_{ib}_{jp}")
                        ebias = 0.0 if exp_bias == 0.0 else eb_t[:]
                        if ws[0] == 512:
                            nc.scalar.activation(pt[:, 0:512 + ws[1]],
                                                 s_ps[:, 0:512 + ws[1]],
                                                 func=AF.Exp, scale=SCALE,
                                                 bias=ebias)
                        else:
                            for half, jb in enumerate(jbs):
                                w = ws[half]
                                nc.scalar.activation(
                                    pt[:, 512 * half:512 * half + w],
                                    s_ps[:, 512 * half:512 * half + w],
                                    func=AF.Exp, scale=SCALE, bias=ebias)
                        for half, jb in enumerate(jbs):
                            w = ws[half]
                            if causal and jb >= 4 * ib:
                                # triangle lives in the first 128 live cols;
                                # beyond that f' >= 128 > p is always kept
                                nc.gpsimd.affine_select(
                                    out=pt[:, 512 * half:512 * half + 128],
                                    in_=pt[:, 512 * half:512 * half + 128],
                                    compare_op=OP.is_ge,
                                    fill=0.0, base=0,
                                    pattern=[[1, 128]], channel_multiplier=-1)
                            nc.tensor.matmul(
                                o_ps[:, 512 - w:512], _r(va[jb][:, h, :]),
                                _r(pt[:, 512 * half:512 * half + w]),
                                start=(jb == 0), stop=(jb == jmax - 1))
                    # normalize: O[d, i] / O[Dh, i]; copy out first so the
                    # psum slot frees before the slow broadcast round-trip
                    ocp = rrp.tile([Dh + 1, 512], F32, tag="ocp",
                                   name=f"ocp{h}_{ib}")
                    nc.vector.tensor_copy(ocp[:], o_ps[:])
                    nc.vector.reciprocal(ocp[Dh:Dh + 1, :], ocp[Dh:Dh + 1, :])
                    rd = rdp.tile([1, 512], F32, tag="rd", name=f"rd{h}_{ib}")
                    nc.sync.dma_start(rd[:], ocp[Dh:Dh + 1, :])
                    r_b = rrp.tile([Dh, 512], F32, tag="rb", name=f"rb{h}_{ib}")
                    bc = bass.AP(tensor=rd[:].tensor, offset=rd[:].offset,
                                 ap=[[0, Dh]] + list(rd[:].ap)[1:])
                    nc.sync.dma_start(r_b[:], bc)
                    if h % 2 == 0:
                        nc.vector.tensor_tensor(oT[p][0:Dh, i0:i0 + 512],
                                                ocp[0:Dh, :], r_b[:],
                                                op=OP.mult)
                    else:
                        stg = rrp.tile([Dh, 512], F32R, tag="stg",
                                       name=f"stg{h}_{ib}")
                        nc.vector.tensor_tensor(stg[:], ocp[0:Dh, :], r_b[:],
                                                op=OP.mult)
                        nc.sync.dma_start(oT[p][Dh:2 * Dh, i0:i0 + 512], stg[:])

        # ---- phase 3: output projection (row-parallel partial) ----
        with tc.tile_pool(name="ob", bufs=4) as obp, \
             tc.tile_pool(name="ps_p", bufs=3, space="PSUM") as ps_p:
            for nb in range(NB):
                n0 = 128 * nb
                pp = ps_p.tile([128, 1024], F32, tag="pp")
                for j2 in range(2):
                    nc.tensor.matmul(pp[:, 512 * j2:512 * (j2 + 1)],
                                     _r(oT[0][:, n0:n0 + 128]),
                                     _r(wp_t[0][:, 512 * j2:512 * (j2 + 1)]),
                                     start=True, stop=False)
                    nc.tensor.matmul(pp[:, 512 * j2:512 * (j2 + 1)],
                                     _r(oT[1][:, n0:n0 + 128]),
                                     _r(wp_t[1][:, 512 * j2:512 * (j2 + 1)]),
                                     start=False, stop=True)
                ob = obp.tile([128, 1024], F32, tag="ob")
                nc.vector.tensor_copy(ob[:], pp[:])
                nc.sync.dma_start(out_e[n0:n0 + 128, :], ob[:])
    return nc


def kernel(x, W_qkv, W_proj, b_proj, ln_g, ln_b, causal, _trace=False):
    global LAST_RESULT
    x = np.asarray(x, dtype=np.float32)
    W_qkv = np.asarray(W_qkv, dtype=np.float32)
    W_proj = np.asarray(W_proj, dtype=np.float32)
    b_proj = np.asarray(b_proj, dtype=np.float32)
    ln_g = np.asarray(ln_g, dtype=np.float32)
    ln_b = np.asarray(ln_b, dtype=np.float32)
    causal = bool(int(np.asarray(causal)))

    fast_gb = bool(np.all(ln_g == 1.0) and np.all(ln_b == 0.0))
    exp_bias = 0.0
    if not fast_gb:
        m = float(SCALE * (8.0 * np.abs(ln_g).max() + 8.0 * np.abs(ln_b).max()) ** 2)
        exp_bias = -max(0.0, m - 8.0)

    key = (causal, fast_gb, exp_bias)
    if key not in _BUILD_CACHE:
        nc = _build(causal, fast_gb, exp_bias)
        nc.finalize()
        _BUILD_CACHE[key] = nc
    nc = _BUILD_CACHE[key]

    xts = [np.ascontiguousarray(x[b].T) for b in range(B)]
    in_maps = []
    for c in range(NCORES):
        b, h0 = c // HPC, Dh * HPC * (c % HPC)   # h0 in channel units
        rows = np.concatenate([W_qkv[s * C + h0: s * C + h0 + HPC * Dh]
                               for s in range(3)])
        im = {
            "xt": xts[b],
            "wqkv_t": np.ascontiguousarray(rows.T),
            "wp_t": np.ascontiguousarray(W_proj[:, h0:h0 + HPC * Dh].T),
        }
        if not fast_gb:
            gseg = np.tile(ln_g, 8)              # q heads x4 then k heads x4
            bseg = np.tile(ln_b, 8)
            im["g_bcast"] = np.broadcast_to(gseg, (128, 512)).copy()
            im["b_bcast"] = np.broadcast_to(bseg, (128, 512)).copy()
        in_maps.append(im)

    res = run_bass_kernel_spmd(nc, in_maps, core_ids=list(range(NCORES)),
                               trace=_trace)
    LAST_RESULT = res

    out = np.empty((B, N, C), dtype=np.float32)
    for b in range(B):
        acc = res.results[4 * b]["out_p"].astype(np.float32).copy()
        for c in range(4 * b + 1, 4 * b + 4):
            acc += res.results[c]["out_p"]
        out[b] = acc + b_proj
    return out



# revision 15
# speedup vs baseline: 1.1393x; 1.1393x over previous
"""Trainium2 Bass kernel for BasicSelfAttention (B=2, N=2048, C=1024, H=16, Dh=64).

Sharding: 8 cores = 2 batches x 4 head-groups. Core c handles batch c//4 and
heads [4*(c%4), 4*(c%4)+4).

v2 design (vs. fp32r baseline):
  - bf16 everywhere on the PE (qkv, scores, PV, proj); fp32 only in PSUM and
    LN statistics. Halves DMA traffic and makes narrow matmul blocks cheap.
  - per-head mean removal via extra weight columns: host appends 8 "row-sum"
    columns to W_qkv so the qkv matmul also produces sum_d(q) per head.
  - all transposes via DMA XBAR (dma transpose), freeing the PE and the
    psum->sbuf copy engines.
  - PV matmul in natural orientation: stationary = P^T block [j,128i],
    moving = v_aug [j,65] -> out [128i, 65] in PSUM at full partition
    utilization (half the PE cost of the [65,w] orientation), and the
    softmax denominator becomes a per-partition scalar: normalize is one
    reciprocal + one broadcast multiply, no DRAM-bounce broadcast.
  - phase-interleaved emission: qkv row-blocks, attention (S/exp/mask),
    PV+normalize, and the output projection are interleaved so PE stays fed
    while ACT grinds through exp.
Host: pre-transposes x and weight slices (bf16), sums the 4 partial
projections per batch, adds b_proj.
"""

import numpy as np
from contextlib import ExitStack

import ml_dtypes
import concourse.bass as bass
import concourse.mybir as mybir
import concourse.tile as tile
from concourse import bacc
from concourse.bass_utils import run_bass_kernel_spmd

B, N, C, H, Dh = 2, 2048, 1024, 16, 64
HPC = 4                      # heads per core
NCORES = 8
SCALE = 8.0 / Dh             # 0.125 (use_mup)
EPS = 1e-5

F32 = mybir.dt.float32
BF16 = mybir.dt.bfloat16
AF = mybir.ActivationFunctionType
OP = mybir.AluOpType
BF = ml_dtypes.bfloat16

NB = N // 128                # 16 row blocks of 128
CB = C // 128                # 8 contraction blocks
IB = N // 512                # 4 query blocks of 512
WQ = 776                     # 256 q | 256 k | 8 head-sums | 256 v

_BUILD_CACHE = {}
LAST_RESULT = None


def _bc3(ap2d, inner):
    """[p, g] AP -> [p, g, inner] with stride-0 inner dim."""
    return bass.AP(tensor=ap2d.tensor, offset=ap2d.offset,
                   ap=list(ap2d.ap) + [[0, inner]])


def _build(causal: bool, fast_gb: bool, exp_bias: float):
    nc = bacc.Bacc("TRN2", target_bir_lowering=False, debug=False,
                   num_devices=NCORES)

    xt_e = nc.dram_tensor("xt", [C, N], BF16, kind="ExternalInput")
    w_e = nc.dram_tensor("w_all", [C, WQ], BF16, kind="ExternalInput")
    wp_e = nc.dram_tensor("wp_t", [HPC * Dh, C], BF16, kind="ExternalInput")
    if not fast_gb:
        gt_e = nc.dram_tensor("g_bcast", [128, 512], F32, kind="ExternalInput")
        bt_e = nc.dram_tensor("b_bcast", [128, 512], F32, kind="ExternalInput")
    out_e = nc.dram_tensor("out_p", [N, C], BF16, kind="ExternalOutput")

    with tile.TileContext(nc) as tc, ExitStack() as ctx:
        persist = ctx.enter_context(tc.tile_pool(name="persist", bufs=1))
        ones_t = persist.tile([128, 1], BF16, tag="ones")
        nc.vector.memset(ones_t[:], 1.0)
        eps_t = persist.tile([128, 1], F32, tag="eps")
        nc.vector.memset(eps_t[:], EPS)
        eb_t = persist.tile([128, 1], F32, tag="ebias")
        nc.vector.memset(eb_t[:], exp_bias)

        # preload the one ACT table that holds Exp+Ln+Copy+Square (id 6 =
        # natural_log_exp_and_others in act_info.json) so the table-load
        # insertion pass never has to switch tables mid-kernel
        nc.scalar.add_instruction(mybir.InstLoadActFuncSet(
            name=nc.get_next_instruction_name(), act_func_set_id=6,
            engine=mybir.EngineType.Activation, ins=[], outs=[]))

        # transposed q|k, segments: 0,1 = q head-pairs; 2,3 = k head-pairs
        qkT = persist.tile([128, 4, N], BF16, tag="qkT")
        # transposed normalized attention output, head-pairs, input to proj
        oT = persist.tile([128, 2, N], BF16, tag="oT")

        xt = [persist.tile([128, N], BF16, tag=f"xt{cb}", name=f"xt{cb}")
              for cb in range(CB)]
        wq = [persist.tile([128, WQ], BF16, tag=f"wq{cb}", name=f"wq{cb}")
              for cb in range(CB)]
        wp_t = [persist.tile([128, C], BF16, tag=f"wp{p}", name=f"wp{p}")
                for p in range(2)]

        if not fast_gb:
            gt = persist.tile([128, 512], F32, tag="gt")
            bt = persist.tile([128, 512], F32, tag="bt")
            nc.sync.dma_start(gt[:], gt_e[:])
            nc.sync.dma_start(bt[:], bt_e[:])

        va_pool = ctx.enter_context(tc.tile_pool(name="va", bufs=NB))
        va = [None] * NB

        ptp = ctx.enter_context(tc.tile_pool(name="pt", bufs=32))
        natp = ctx.enter_context(tc.tile_pool(name="nat", bufs=3))
        sqp = ctx.enter_context(tc.tile_pool(name="sq", bufs=3))
        stp = ctx.enter_context(tc.tile_pool(name="st", bufs=16))
        osp = ctx.enter_context(tc.tile_pool(name="os", bufs=3))
        obp = ctx.enter_context(tc.tile_pool(name="ob", bufs=3))
        # PSUM: qp serves qkv blocks + proj halves; sp serves S tiles + o tiles
        qp = ctx.enter_context(tc.tile_pool(name="qp", bufs=2, space="PSUM"))
        sp = ctx.enter_context(tc.tile_pool(name="sp", bufs=2, space="PSUM"))

        # ---- input DMAs: weights + first column wave first ----
        for cb in range(CB):
            nc.sync.dma_start(wq[cb][:], w_e[128 * cb:128 * (cb + 1), :])
            nc.sync.dma_start(xt[cb][:, 0:256],
                              xt_e[128 * cb:128 * (cb + 1), 0:256])
        for cb in range(CB):
            nc.sync.dma_start(xt[cb][:, 256:N],
                              xt_e[128 * cb:128 * (cb + 1), 256:N])
        for p in range(2):
            nc.sync.dma_start(wp_t[p][:], wp_e[128 * p:128 * (p + 1), :])

        # ---- qkv + LN + transposes + v_aug for one 128-row block ----
        def emit_nb(nb):
            n0 = 128 * nb
            qps = qp.tile([128, WQ], F32, tag="qkv", name=f"qkv{nb}")
            for cb in range(CB):
                st = (cb == 0)
                spf = (cb == CB - 1)
                nc.tensor.matmul(qps[:, 0:512], xt[cb][:, n0:n0 + 128],
                                 wq[cb][:, 0:512], start=st, stop=spf)
                nc.tensor.matmul(qps[:, 512:WQ], xt[cb][:, n0:n0 + 128],
                                 wq[cb][:, 512:WQ], start=st, stop=spf)
            nat = natp.tile([128, WQ], BF16, tag="nat", name=f"nat{nb}")
            nc.vector.tensor_copy(nat[:], qps[:])
            sq = sqp.tile([128, 512], BF16, tag="sq", name=f"sq{nb}")
            nc.vector.tensor_tensor(sq[:], nat[:, 0:512], nat[:, 0:512],
                                    op=OP.mult)

            mean = stp.tile([128, 8], F32, tag="mean", name=f"mean{nb}")
            nc.vector.tensor_scalar(mean[:], qps[:, 512:520], 1.0 / Dh, None,
                                    op0=OP.mult)
            sqs = stp.tile([128, 8], F32, tag="sqs", name=f"sqs{nb}")
            nc.vector.tensor_reduce(sqs[:],
                                    sq[:].rearrange("p (g d) -> p g d", g=8),
                                    axis=mybir.AxisListType.X, op=OP.add)
            msq = stp.tile([128, 8], F32, tag="msq", name=f"msq{nb}")
            nc.vector.tensor_tensor(msq[:], mean[:], mean[:], op=OP.mult)
            rstd = stp.tile([128, 8], F32, tag="rstd", name=f"rstd{nb}")
            nc.vector.scalar_tensor_tensor(rstd[:], sqs[:], 1.0 / Dh, msq[:],
                                           op0=OP.mult, op1=OP.subtract)
            # rstd = (var+eps)^-0.5 = exp(-0.5*ln(var+eps)); Ln/Exp share an
            # ACT table with the softmax Exp, so no act-table reloads
            nc.scalar.activation(rstd[:], rstd[:], func=AF.Ln, bias=eps_t[:])
            nc.scalar.activation(rstd[:], rstd[:], func=AF.Exp, scale=-0.5)

            qk3 = nat[:, 0:512].rearrange("p (g d) -> p g d", g=8)
            nc.vector.tensor_tensor(qk3, qk3, _bc3(mean[:], Dh),
                                    op=OP.subtract)
            nc.vector.tensor_tensor(qk3, qk3, _bc3(rstd[:], Dh), op=OP.mult)
            if not fast_gb:
                nc.vector.tensor_tensor(nat[:, 0:512], nat[:, 0:512], gt[:],
                                        op=OP.mult)
                nc.vector.tensor_tensor(nat[:, 0:512], nat[:, 0:512], bt[:],
                                        op=OP.add)

            nc.sync.dma_start(qkT[:, :, n0:n0 + 128], nat[:, 0:512],
                              transpose=True)

            vat = va_pool.tile([128, HPC, Dh + 1], BF16, tag="vat",
                               name=f"vat{nb}")
            nc.gpsimd.tensor_copy(
                vat[:, :, 0:Dh],
                nat[:, 520:776].rearrange("p (h d) -> p h d", h=HPC))
            nc.gpsimd.tensor_copy(vat[:, :, Dh:Dh + 1],
                                  ones_t[:].to_broadcast([128, HPC, 1]))
            va[nb] = vat

        def width(ib, jb):
            if not causal or jb < 4 * ib:
                return 512
            return 512 - 128 * (jb - 4 * ib)

        # ---- S + exp + mask for one (ib, h, jp); returns the pt tile ----
        def emit_sjp(ib, h, jp):
            p, off = h // 2, 64 * (h % 2)
            i0 = 512 * ib
            jbs = (2 * jp, 2 * jp + 1)
            ws = [width(ib, jb) for jb in jbs]
            # storage: half0 right-aligned to 512, half1 left-aligned at 512
            # -> live cols [512-ws0, 512+ws1) always contiguous
            s_ps = sp.tile([128, 1024], F32, tag="sps",
                           name=f"s{ib}_{h}_{jp}")
            c0s = [512 - ws[0], 512]
            for half, jb in enumerate(jbs):
                w = ws[half]
                nc.tensor.matmul(
                    s_ps[:, c0s[half]:c0s[half] + w],
                    qkT[off:off + Dh, 2 + p, 128 * jb:128 * (jb + 1)],
                    qkT[off:off + Dh, p, i0 + 512 - w:i0 + 512],
                    start=True, stop=True)
            pt = ptp.tile([128, 1024], BF16, tag="pt",
                          name=f"pt{ib}_{h}_{jp}")
            ebias = 0.0 if exp_bias == 0.0 else eb_t[:]
            nc.scalar.activation(pt[:, c0s[0]:512 + ws[1]],
                                 s_ps[:, c0s[0]:512 + ws[1]],
                                 func=AF.Exp, scale=SCALE, bias=ebias)
            for half, jb in enumerate(jbs):
                if causal and jb >= 4 * ib:
                    # triangle lives in the first live 128 cols of this half
                    t = 128 * (jb - 4 * ib)
                    c = (t if half == 0 else 512)
                    nc.gpsimd.affine_select(
                        out=pt[:, c:c + 128], in_=pt[:, c:c + 128],
                        compare_op=OP.is_ge, fill=0.0, base=0,
                        pattern=[[1, 128]], channel_multiplier=-1)
            return pt

        # pt column of block jb for query chunk b (global 128-chunk index)
        def pt_col(ib, jb, half, b):
            bp = b - 4 * ib
            if half == 0:
                return 128 * bp          # right-aligned or full: col = 128*bp
            return 512 + 128 * bp - (512 - width(ib, jb))

        # ---- PV + normalize + O-transpose for all 4 i-chunks of ib ----
        def emit_pv(ib, pts):
            for bp in range(4):
                g = 4 * ib + bp
                jmax = g + 1 if causal else NB
                o_t = qp.tile([128, WQ], F32, tag="qkv",
                              name=f"o{ib}_{bp}")
                o_ps = o_t[:, 0:HPC * (Dh + 1)].rearrange(
                    "p (h d) -> p h d", h=HPC)
                for h in range(HPC):
                    for jb in range(jmax):
                        jp, half = jb // 2, jb % 2
                        col = pt_col(ib, jb, half, g)
                        pt = pts[(h, jp)]
                        nc.tensor.matmul(
                            o_ps[:, h, :], pt[:, col:col + 128],
                            va[jb][:, h, :],
                            start=(jb == 0), stop=(jb == jmax - 1))
                rd = stp.tile([128, HPC, 1], F32, tag="rd",
                              name=f"rd{ib}_{bp}")
                nc.vector.reciprocal(rd[:], o_ps[:, :, Dh:Dh + 1])
                osb = osp.tile([128, HPC, Dh], BF16, tag="osb",
                               name=f"osb{ib}_{bp}")
                nc.vector.tensor_tensor(osb[:], o_ps[:, :, 0:Dh],
                                        _bc3(rd[:, :, 0], Dh), op=OP.mult)
                n0 = 128 * g
                nc.sync.dma_start(oT[:, :, n0:n0 + 128], osb[:],
                                  transpose=True)

        # ---- output projection for one 128-row block ----
        def emit_proj(nb):
            n0 = 128 * nb
            ob = obp.tile([128, C], BF16, tag="ob", name=f"ob{nb}")
            for j2 in range(2):
                pp_t = qp.tile([128, WQ], F32, tag="qkv", name=f"pp{nb}_{j2}")
                pp = pp_t[:, 0:512]
                nc.tensor.matmul(pp[:], oT[:, 0, n0:n0 + 128],
                                 wp_t[0][:, 512 * j2:512 * (j2 + 1)],
                                 start=True, stop=False)
                nc.tensor.matmul(pp[:], oT[:, 1, n0:n0 + 128],
                                 wp_t[1][:, 512 * j2:512 * (j2 + 1)],
                                 start=False, stop=True)
                if j2 == 0:
                    nc.vector.tensor_copy(ob[:, 0:512], pp[:])
                else:
                    nc.scalar.activation(ob[:, 512:1024], pp[:], func=AF.Copy)
            nc.sync.dma_start(out_e[n0:n0 + 128, :], ob[:])

        # ---- interleaved emission ----
        def interleave(units, extras):
            if not extras:
                for u in units:
                    u()
                return
            k = len(units) / (len(extras) + 1)
            nxt, ei = k, 0
            for i, u in enumerate(units):
                u()
                while ei < len(extras) and i + 1 >= nxt:
                    extras[ei]()
                    ei += 1
                    nxt += k
            while ei < len(extras):
                extras[ei]()
                ei += 1

        # proj(ib) is PE-dense/ACT-light: bank it as filler for the later,
        # exp-heavy attention sections (ib3's exp alone is ~26us of ACT).
        for nb in range(4):
            emit_nb(nb)
        proj_sched = {1: [], 2: [0], 3: [1, 2]}   # section -> prior ibs to proj
        for k in range(IB):
            pts = {}
            units = []
            jmax = 4 * (k + 1) if causal else NB
            for h in range(HPC):
                for jp in range(jmax // 2):
                    units.append(lambda ib=k, h=h, jp=jp:
                                 pts.__setitem__((h, jp), emit_sjp(ib, h, jp)))
            extras = []
            if k < IB - 1:
                extras += [lambda nb=nb: emit_nb(nb)
                           for nb in range(4 * (k + 1), 4 * (k + 2))]
            for pib in proj_sched.get(k, []):
                extras += [lambda nb=nb: emit_proj(nb)
                           for nb in range(4 * pib, 4 * pib + 4)]
            interleave(units, extras)
            emit_pv(k, pts)
        for nb in range(4 * (IB - 1), 4 * IB):
            emit_proj(nb)
    return nc


def kernel(x, W_qkv, W_proj, b_proj, ln_g, ln_b, causal, _trace=False):
    global LAST_RESULT
    x = np.asarray(x, dtype=np.float32)
    W_qkv = np.asarray(W_qkv, dtype=np.float32)
    W_proj = np.asarray(W_proj, dtype=np.float32)
    b_proj = np.asarray(b_proj, dtype=np.float32)
    ln_g = np.asarray(ln_g, dtype=np.float32)
    ln_b = np.asarray(ln_b, dtype=np.float32)
    causal = bool(int(np.asarray(causal)))

    fast_gb = bool(np.all(ln_g == 1.0) and np.all(ln_b == 0.0))
    exp_bias = 0.0
    if not fast_gb:
        m = float(SCALE * (8.0 * np.abs(ln_g).max() + 8.0 * np.abs(ln_b).max()) ** 2)
        exp_bias = -max(0.0, m - 8.0)

    key = (causal, fast_gb, exp_bias)
    if key not in _BUILD_CACHE:
        nc = _build(causal, fast_gb, exp_bias)
        nc.finalize()
        _BUILD_CACHE[key] = nc
    nc = _BUILD_CACHE[key]

    xts = [np.ascontiguousarray(x[b].T).astype(BF) for b in range(B)]
    in_maps = []
    for c in range(NCORES):
        b, h0 = c // HPC, Dh * HPC * (c % HPC)   # h0 in channel units
        rq = W_qkv[h0:h0 + 256]
        rk = W_qkv[C + h0:C + h0 + 256]
        rv = W_qkv[2 * C + h0:2 * C + h0 + 256]
        sums = np.concatenate(
            [rq.reshape(4, Dh, C).sum(axis=1), rk.reshape(4, Dh, C).sum(axis=1)])
        w_all = np.concatenate([rq, rk, sums, rv])        # [776, 1024]
        im = {
            "xt": xts[b],
            "w_all": np.ascontiguousarray(w_all.T).astype(BF),
            "wp_t": np.ascontiguousarray(W_proj[:, h0:h0 + 256].T).astype(BF),
        }
        if not fast_gb:
            gseg = np.tile(ln_g, 8)              # q heads x4 then k heads x4
            bseg = np.tile(ln_b, 8)
            im["g_bcast"] = np.broadcast_to(gseg, (128, 512)).copy()
            im["b_bcast"] = np.broadcast_to(bseg, (128, 512)).copy()
        in_maps.append(im)

    res = run_bass_kernel_spmd(nc, in_maps, core_ids=list(range(NCORES)),
                               trace=_trace)
    LAST_RESULT = res

    out = np.empty((B, N, C), dtype=np.float32)
    for b in range(B):
        acc = res.results[4 * b]["out_p"].astype(np.float32)
        for c in range(4 * b + 1, 4 * b + 4):
            acc = acc + res.results[c]["out_p"].astype(np.float32)
        out[b] = acc + b_proj
    return out


# revision 53
# speedup vs baseline: 1.1956x; 1.0495x over previous
"""Trainium2 Bass kernel for BasicSelfAttention (B=2, N=2048, C=1024, H=16, Dh=64).

Sharding: 8 cores = 2 batches x 4 head-groups. Core c handles batch c//4 and
heads [4*(c%4), 4*(c%4)+4).

v2 design (vs. the fp32r baseline):
  - bf16 everywhere on the PE (qkv, scores, PV, proj); fp32 only in PSUM and
    LN statistics. Halves DMA traffic and makes narrow matmul blocks cheap.
  - per-head mean removal via extra weight columns: host appends 8 "row-sum"
    columns to W_qkv so the qkv matmul also produces sum_d(q) per head.
  - rstd = exp(-0.5*ln(var+eps)): Ln/Exp/Copy/Square share one ACT table
    (preloaded once as id 6), so no activation-table reloads ever happen.
  - all transposes via DMA XBAR (dma transpose, s-major row mapping verified
    on HW), freeing the PE and the psum->sbuf copy engines.
  - PV matmul in natural orientation: stationary = P^T block [j,128i],
    moving = v_aug [j,65] -> out [128i, 65] in PSUM at full partition
    utilization (half the PE cost of the [65,w] orientation), and the
    softmax denominator becomes a per-partition scalar: normalize is one
    reciprocal + one broadcast multiply, no DRAM-bounce broadcast.
  - phase-interleaved emission: qkv row-blocks, attention (S/exp/mask),
    PV+normalize, and the output projection are interleaved so PE stays fed
    while ACT grinds through exp.
Host: pre-transposes x and weight slices (bf16), sums the 4 partial
projections per batch, adds b_proj.
"""

import numpy as np
from contextlib import ExitStack

import ml_dtypes
import concourse.bass as bass
import concourse.mybir as mybir
import concourse.tile as tile
from concourse import bacc
from concourse.bass_utils import run_bass_kernel_spmd

B, N, C, H, Dh = 2, 2048, 1024, 16, 64
HPC = 4                      # heads per core
NCORES = 8
SCALE = 8.0 / Dh             # 0.125 (use_mup)
EPS = 1e-5

F32 = mybir.dt.float32
BF16 = mybir.dt.bfloat16
AF = mybir.ActivationFunctionType
OP = mybir.AluOpType
BF = ml_dtypes.bfloat16

NB = N // 128                # 16 row blocks of 128
CB = C // 128                # 8 contraction blocks
IB = N // 512                # 4 query blocks of 512
WQ = 776                     # 256 q | 256 k | 8 head-sums | 256 v

_BUILD_CACHE = {}
LAST_RESULT = None


def _bc3(ap2d, inner):
    """[p, g] AP -> [p, g, inner] with stride-0 inner dim."""
    return bass.AP(tensor=ap2d.tensor, offset=ap2d.offset,
                   ap=list(ap2d.ap) + [[0, inner]])


def _build(causal: bool, fast_gb: bool, exp_bias: float):
    nc = bacc.Bacc("TRN2", target_bir_lowering=False, debug=False,
                   num_devices=NCORES)

    xt_e = nc.dram_tensor("xt", [C, N], BF16, kind="ExternalInput")
    w_e = nc.dram_tensor("w_all", [C, WQ], BF16, kind="ExternalInput")
    wp_e = nc.dram_tensor("wp_t", [HPC * Dh, C], BF16, kind="ExternalInput")
    if not fast_gb:
        gt_e = nc.dram_tensor("g_bcast", [128, 512], F32, kind="ExternalInput")
        bt_e = nc.dram_tensor("b_bcast", [128, 512], F32, kind="ExternalInput")
    out_e = nc.dram_tensor("out_p", [N, C], BF16, kind="ExternalOutput")

    with tile.TileContext(nc) as tc, ExitStack() as ctx:
        persist = ctx.enter_context(tc.tile_pool(name="persist", bufs=1))
        ones_t = persist.tile([128, 1], BF16, tag="ones")
        nc.vector.memset(ones_t[:], 1.0)
        eps_t = persist.tile([128, 1], F32, tag="eps")
        nc.vector.memset(eps_t[:], EPS)
        eb_t = persist.tile([128, 1], F32, tag="ebias")
        nc.vector.memset(eb_t[:], exp_bias)

        # preload the one ACT table that holds Exp+Ln+Copy+Square (id 6 =
        # natural_log_exp_and_others in act_info.json) so the table-load
        # insertion pass never has to switch tables mid-kernel
        nc.scalar.add_instruction(mybir.InstLoadActFuncSet(
            name=nc.get_next_instruction_name(), act_func_set_id=6,
            engine=mybir.EngineType.Activation, ins=[], outs=[]))

        # transposed q|k, segments: 0,1 = q head-pairs; 2,3 = k head-pairs
        qkT = persist.tile([128, 4, N], BF16, tag="qkT")
        # transposed normalized attention output, head-pairs, input to proj
        oT = persist.tile([128, 2, N], BF16, tag="oT")

        xt = [persist.tile([128, N], BF16, tag=f"xt{cb}", name=f"xt{cb}")
              for cb in range(CB)]
        wq = [persist.tile([128, WQ], BF16, tag=f"wq{cb}", name=f"wq{cb}")
              for cb in range(CB)]
        wp_t = [persist.tile([128, C], BF16, tag=f"wp{p}", name=f"wp{p}")
                for p in range(2)]

        if not fast_gb:
            gt = persist.tile([128, 512], F32, tag="gt")
            bt = persist.tile([128, 512], F32, tag="bt")
            nc.sync.dma_start(gt[:], gt_e[:])
            nc.sync.dma_start(bt[:], bt_e[:])

        va_pool = ctx.enter_context(tc.tile_pool(name="va", bufs=NB))
        va = [None] * NB

        ptp = ctx.enter_context(tc.tile_pool(name="pt", bufs=32))
        natp = ctx.enter_context(tc.tile_pool(name="nat", bufs=4))
        sqp = ctx.enter_context(tc.tile_pool(name="sq", bufs=4))
        stp = ctx.enter_context(tc.tile_pool(name="st", bufs=24))
        osp = ctx.enter_context(tc.tile_pool(name="os", bufs=4))
        obp = ctx.enter_context(tc.tile_pool(name="ob", bufs=4))
        # PSUM: qp serves qkv blocks + proj halves; sp serves S tiles + o
        qp = ctx.enter_context(tc.tile_pool(name="qp", bufs=2, space="PSUM"))
        sp = ctx.enter_context(tc.tile_pool(name="sp", bufs=2, space="PSUM"))

        # ---- input DMAs: weights + first column wave first ----
        for cb in range(CB):
            nc.sync.dma_start(wq[cb][:], w_e[128 * cb:128 * (cb + 1), :])
            nc.sync.dma_start(xt[cb][:, 0:256],
                              xt_e[128 * cb:128 * (cb + 1), 0:256])
        for cb in range(CB):
            nc.sync.dma_start(xt[cb][:, 256:N],
                              xt_e[128 * cb:128 * (cb + 1), 256:N])
        for p in range(2):
            nc.sync.dma_start(wp_t[p][:], wp_e[128 * p:128 * (p + 1), :])

        # ---- qkv + LN + transposes + v_aug for one 128-row block ----
        def emit_nb(nb):
            n0 = 128 * nb
            qps = qp.tile([128, WQ], F32, tag="qkv", name=f"qkv{nb}")
            for cb in range(CB):
                st = (cb == 0)
                spf = (cb == CB - 1)
                nc.tensor.matmul(qps[:, 0:512], xt[cb][:, n0:n0 + 128],
                                 wq[cb][:, 0:512], start=st, stop=spf)
                nc.tensor.matmul(qps[:, 512:WQ], xt[cb][:, n0:n0 + 128],
                                 wq[cb][:, 512:WQ], start=st, stop=spf)
            nat = natp.tile([128, WQ], BF16, tag="nat", name=f"nat{nb}")
            nc.vector.tensor_copy(nat[:], qps[:])
            sq = sqp.tile([128, 512], BF16, tag="sq", name=f"sq{nb}")
            nc.vector.tensor_tensor(sq[:], nat[:, 0:512], nat[:, 0:512],
                                    op=OP.mult)

            mean = stp.tile([128, 8], F32, tag="mean", name=f"mean{nb}")
            nc.vector.tensor_scalar(mean[:], qps[:, 512:520], 1.0 / Dh, None,
                                    op0=OP.mult)
            sqs = stp.tile([128, 8], F32, tag="sqs", name=f"sqs{nb}")
            nc.vector.tensor_reduce(sqs[:],
                                    sq[:].rearrange("p (g d) -> p g d", g=8),
                                    axis=mybir.AxisListType.X, op=OP.add)
            msq = stp.tile([128, 8], F32, tag="msq", name=f"msq{nb}")
            nc.vector.tensor_tensor(msq[:], mean[:], mean[:], op=OP.mult)
            rstd = stp.tile([128, 8], F32, tag="rstd", name=f"rstd{nb}")
            nc.vector.scalar_tensor_tensor(rstd[:], sqs[:], 1.0 / Dh, msq[:],
                                           op0=OP.mult, op1=OP.subtract)
            # rstd = (var+eps)^-0.5 = exp(-0.5*ln(var+eps)); Ln/Exp share an
            # ACT table with the softmax Exp, so no act-table reloads
            nc.scalar.activation(rstd[:], rstd[:], func=AF.Ln, bias=eps_t[:])
            nc.scalar.activation(rstd[:], rstd[:], func=AF.Exp, scale=-0.5)

            qk3 = nat[:, 0:512].rearrange("p (g d) -> p g d", g=8)
            nc.vector.tensor_tensor(qk3, qk3, _bc3(mean[:], Dh),
                                    op=OP.subtract)
            nc.vector.tensor_tensor(qk3, qk3, _bc3(rstd[:], Dh), op=OP.mult)
            if not fast_gb:
                nc.vector.tensor_tensor(nat[:, 0:512], nat[:, 0:512], gt[:],
                                        op=OP.mult)
                nc.vector.tensor_tensor(nat[:, 0:512], nat[:, 0:512], bt[:],
                                        op=OP.add)

            nc.sync.dma_start(qkT[:, :, n0:n0 + 128], nat[:, 0:512],
                              transpose=True)

            vat = va_pool.tile([128, HPC, Dh + 1], BF16, tag="vat",
                               name=f"vat{nb}")
            nc.gpsimd.tensor_copy(
                vat[:, :, 0:Dh],
                nat[:, 520:776].rearrange("p (h d) -> p h d", h=HPC))
            nc.gpsimd.tensor_copy(vat[:, :, Dh:Dh + 1],
                                  ones_t[:].to_broadcast([128, HPC, 1]))
            va[nb] = vat

        def width(ib, jb):
            if not causal or jb < 4 * ib:
                return 512
            return 512 - 128 * (jb - 4 * ib)

        # ---- S + exp + mask for one (ib, h, jp); returns the pt tile ----
        def emit_sjp(ib, h, jp):
            p, off = h // 2, 64 * (h % 2)
            i0 = 512 * ib
            jbs = (2 * jp, 2 * jp + 1)
            ws = [width(ib, jb) for jb in jbs]
            # storage: half0 right-aligned to 512, half1 left-aligned at 512
            # -> live cols [512-ws0, 512+ws1) always contiguous
            s_ps = sp.tile([128, 1024], F32, tag="sps",
                           name=f"s{ib}_{h}_{jp}")
            c0s = [512 - ws[0], 512]
            for half, jb in enumerate(jbs):
                w = ws[half]
                nc.tensor.matmul(
                    s_ps[:, c0s[half]:c0s[half] + w],
                    qkT[off:off + Dh, 2 + p, 128 * jb:128 * (jb + 1)],
                    qkT[off:off + Dh, p, i0 + 512 - w:i0 + 512],
                    start=True, stop=True)
            pt = ptp.tile([128, 1024], BF16, tag="pt",
                          name=f"pt{ib}_{h}_{jp}")
            ebias = 0.0 if exp_bias == 0.0 else eb_t[:]
            nc.scalar.activation(pt[:, c0s[0]:512 + ws[1]],
                                 s_ps[:, c0s[0]:512 + ws[1]],
                                 func=AF.Exp, scale=SCALE, bias=ebias)
            for half, jb in enumerate(jbs):
                if causal and jb >= 4 * ib:
                    # triangle lives in the first live 128 cols of this half
                    t = 128 * (jb - 4 * ib)
                    c = (t if half == 0 else 512)
                    nc.gpsimd.affine_select(
                        out=pt[:, c:c + 128], in_=pt[:, c:c + 128],
                        compare_op=OP.is_ge, fill=0.0, base=0,
                        pattern=[[1, 128]], channel_multiplier=-1)
            return pt

        # pt column of block jb for query chunk g (global 128-chunk index)
        def pt_col(ib, jb, half, g):
            bp = g - 4 * ib
            if half == 0:
                return 128 * bp          # right-aligned or full: col = 128*bp
            return 512 + 128 * bp - (512 - width(ib, jb))

        # ---- PV + normalize + O-transpose for all 4 i-chunks of ib ----
        def emit_pv(ib, pts):
            for bp in range(4):
                g = 4 * ib + bp
                jmax = g + 1 if causal else NB
                o_t = sp.tile([128, 1024], F32, tag="sps",
                              name=f"o{ib}_{bp}")
                o_ps = o_t[:, 0:HPC * (Dh + 1)].rearrange(
                    "p (h d) -> p h d", h=HPC)
                for h in range(HPC):
                    for jb in range(jmax):
                        jp, half = jb // 2, jb % 2
                        col = pt_col(ib, jb, half, g)
                        pt = pts[(h, jp)]
                        nc.tensor.matmul(
                            o_ps[:, h, :], pt[:, col:col + 128],
                            va[jb][:, h, :],
                            start=(jb == 0), stop=(jb == jmax - 1))
                rd = stp.tile([128, HPC, 1], F32, tag="rd",
                              name=f"rd{ib}_{bp}")
                nc.vector.reciprocal(rd[:], o_ps[:, :, Dh:Dh + 1])
                osb = osp.tile([128, HPC, Dh], BF16, tag="osb",
                               name=f"osb{ib}_{bp}")
                nc.vector.tensor_tensor(osb[:], o_ps[:, :, 0:Dh],
                                        _bc3(rd[:, :, 0], Dh), op=OP.mult)
                n0 = 128 * g
                nc.sync.dma_start(oT[:, :, n0:n0 + 128], osb[:],
                                  transpose=True)

        # ---- output projection for one 128-row block ----
        def emit_proj(nb):
            n0 = 128 * nb
            ob = obp.tile([128, C], BF16, tag="ob", name=f"ob{nb}")
            for j2 in range(2):
                pp_t = qp.tile([128, WQ], F32, tag="qkv", name=f"pp{nb}_{j2}")
                pp = pp_t[:, 0:512]
                nc.tensor.matmul(pp[:], oT[:, 0, n0:n0 + 128],
                                 wp_t[0][:, 512 * j2:512 * (j2 + 1)],
                                 start=True, stop=False)
                nc.tensor.matmul(pp[:], oT[:, 1, n0:n0 + 128],
                                 wp_t[1][:, 512 * j2:512 * (j2 + 1)],
                                 start=False, stop=True)
                nc.vector.tensor_copy(ob[:, 512 * j2:512 * (j2 + 1)], pp[:])
            nc.sync.dma_start(out_e[n0:n0 + 128, :], ob[:])

        # ---- interleaved emission ----
        def interleave(units, extras):
            if not extras:
                for u in units:
                    u()
                return
            k = len(units) / (len(extras) + 1)
            nxt, ei = k * 2.0, 0
            for i, u in enumerate(units):
                u()
                while ei < len(extras) and i + 1 >= nxt:
                    extras[ei]()
                    ei += 1
                    nxt += k
            while ei < len(extras):
                extras[ei]()
                ei += 1

        for nb in range(4):
            emit_nb(nb)
        proj_sched = {3: [0, 1, 2]}
        for k in range(IB):
            pts = {}
            units = []
            jmax = 4 * (k + 1) if causal else NB
            for h in range(HPC):
                for jp in range(jmax // 2):
                    units.append(lambda ib=k, h=h, jp=jp:
                                 pts.__setitem__((h, jp), emit_sjp(ib, h, jp)))
            extras = []
            if k < IB - 1:
                extras += [lambda nb=nb: emit_nb(nb)
                           for nb in range(4 * (k + 1), 4 * (k + 2))]
            for pib in proj_sched.get(k, []):
                extras += [lambda nb=nb: emit_proj(nb)
                           for nb in range(4 * pib, 4 * pib + 4)]
            interleave(units, extras)
            emit_pv(k, pts)
        for nb in range(4 * (IB - 1), 4 * IB):
            emit_proj(nb)
    return nc


def kernel(x, W_qkv, W_proj, b_proj, ln_g, ln_b, causal, _trace=False):
    global LAST_RESULT
    x = np.asarray(x, dtype=np.float32)
    W_qkv = np.asarray(W_qkv, dtype=np.float32)
    W_proj = np.asarray(W_proj, dtype=np.float32)
    b_proj = np.asarray(b_proj, dtype=np.float32)
    ln_g = np.asarray(ln_g, dtype=np.float32)
    ln_b = np.asarray(ln_b, dtype=np.float32)
    causal = bool(int(np.asarray(causal)))

    fast_gb = bool(np.all(ln_g == 1.0) and np.all(ln_b == 0.0))
    exp_bias = 0.0
    if not fast_gb:
        m = float(SCALE * (8.0 * np.abs(ln_g).max() + 8.0 * np.abs(ln_b).max()) ** 2)
        exp_bias = -max(0.0, m - 8.0)

    key = (causal, fast_gb, exp_bias)
    if key not in _BUILD_CACHE:
        nc = _build(causal, fast_gb, exp_bias)
        nc.finalize()
        _BUILD_CACHE[key] = nc
    nc = _BUILD_CACHE[key]

    xts = [np.ascontiguousarray(x[b].T).astype(BF) for b in range(B)]
    in_maps = []
    for c in range(NCORES):
        b, h0 = c // HPC, Dh * HPC * (c % HPC)   # h0 in channel units
        rq = W_qkv[h0:h0 + 256]
        rk = W_qkv[C + h0:C + h0 + 256]
        rv = W_qkv[2 * C + h0:2 * C + h0 + 256]
        sums = np.concatenate(
            [rq.reshape(4, Dh, C).sum(axis=1), rk.reshape(4, Dh, C).sum(axis=1)])
        w_all = np.concatenate([rq, rk, sums, rv])        # [776, 1024]
        im = {
            "xt": xts[b],
            "w_all": np.ascontiguousarray(w_all.T).astype(BF),
            "wp_t": np.ascontiguousarray(W_proj[:, h0:h0 + 256].T).astype(BF),
        }
        if not fast_gb:
            gseg = np.tile(ln_g, 8)              # q heads x4 then k heads x4
            bseg = np.tile(ln_b, 8)
            im["g_bcast"] = np.broadcast_to(gseg, (128, 512)).copy()
            im["b_bcast"] = np.broadcast_to(bseg, (128, 512)).copy()
        in_maps.append(im)

    res = run_bass_kernel_spmd(nc, in_maps, core_ids=list(range(NCORES)),
                               trace=_trace)
    LAST_RESULT = res

    out = np.empty((B, N, C), dtype=np.float32)
    for b in range(B):
        acc = res.results[4 * b]["out_p"].astype(np.float32)
        for c in range(4 * b + 1, 4 * b + 4):
            acc = acc + res.results[c]["out_p"].astype(np.float32)
        out[b] = acc + b_proj
    return out


# revision 56
# speedup vs baseline: 1.2037x; 1.0068x over previous
"""Trainium2 Bass kernel for BasicSelfAttention (B=2, N=2048, C=1024, H=16, Dh=64).

Sharding: 8 cores = 2 batches x 4 head-groups. Core c handles batch c//4 and
heads [4*(c%4), 4*(c%4)+4).

v2 design (vs. the fp32r baseline):
  - bf16 everywhere on the PE (qkv, scores, PV, proj); fp32 only in PSUM and
    LN statistics. Halves DMA traffic and makes narrow matmul blocks cheap.
  - per-head mean removal via extra weight columns: host appends 8 "row-sum"
    columns to W_qkv so the qkv matmul also produces sum_d(q) per head.
  - rstd = exp(-0.5*ln(var+eps)): Ln/Exp/Copy/Square share one ACT table
    (preloaded once as id 6), so no activation-table reloads ever happen.
  - all transposes via DMA XBAR (dma transpose, s-major row mapping verified
    on HW), freeing the PE and the psum->sbuf copy engines.
  - PV matmul in natural orientation: stationary = P^T block [j,128i],
    moving = v_aug [j,65] -> out [128i, 65] in PSUM at full partition
    utilization (half the PE cost of the [65,w] orientation), and the
    softmax denominator becomes a per-partition scalar: normalize is one
    reciprocal + one broadcast multiply, no DRAM-bounce broadcast.
  - phase-interleaved emission: qkv row-blocks, attention (S/exp/mask),
    PV+normalize, and the output projection are interleaved so PE stays fed
    while ACT grinds through exp.
Host: pre-transposes x and weight slices (bf16), sums the 4 partial
projections per batch, adds b_proj.
"""

import numpy as np
from contextlib import ExitStack

import ml_dtypes
import concourse.bass as bass
import concourse.mybir as mybir
import concourse.tile as tile
from concourse import bacc
from concourse.bass_utils import run_bass_kernel_spmd

B, N, C, H, Dh = 2, 2048, 1024, 16, 64
HPC = 4                      # heads per core
NCORES = 8
SCALE = 8.0 / Dh             # 0.125 (use_mup)
EPS = 1e-5

F32 = mybir.dt.float32
BF16 = mybir.dt.bfloat16
AF = mybir.ActivationFunctionType
OP = mybir.AluOpType
BF = ml_dtypes.bfloat16

NB = N // 128                # 16 row blocks of 128
CB = C // 128                # 8 contraction blocks
IB = N // 512                # 4 query blocks of 512
WQ = 776                     # 256 q | 256 k | 8 head-sums | 256 v

_BUILD_CACHE = {}
LAST_RESULT = None


def _bc3(ap2d, inner):
    """[p, g] AP -> [p, g, inner] with stride-0 inner dim."""
    return bass.AP(tensor=ap2d.tensor, offset=ap2d.offset,
                   ap=list(ap2d.ap) + [[0, inner]])


def _build(causal: bool, fast_gb: bool, exp_bias: float):
    nc = bacc.Bacc("TRN2", target_bir_lowering=False, debug=False,
                   num_devices=NCORES)

    xt_e = nc.dram_tensor("xt", [C, N], BF16, kind="ExternalInput")
    w_e = nc.dram_tensor("w_all", [C, WQ], BF16, kind="ExternalInput")
    wp_e = nc.dram_tensor("wp_t", [HPC * Dh, C], BF16, kind="ExternalInput")
    if not fast_gb:
        gt_e = nc.dram_tensor("g_bcast", [128, 512], F32, kind="ExternalInput")
        bt_e = nc.dram_tensor("b_bcast", [128, 512], F32, kind="ExternalInput")
    out_e = nc.dram_tensor("out_p", [N, C], BF16, kind="ExternalOutput")

    with tile.TileContext(nc) as tc, ExitStack() as ctx:
        persist = ctx.enter_context(tc.tile_pool(name="persist", bufs=1))
        ones_t = persist.tile([128, 1], BF16, tag="ones")
        nc.vector.memset(ones_t[:], 1.0)
        eps_t = persist.tile([128, 1], F32, tag="eps")
        nc.vector.memset(eps_t[:], EPS)
        eb_t = persist.tile([128, 1], F32, tag="ebias")
        nc.vector.memset(eb_t[:], exp_bias)

        # preload the one ACT table that holds Exp+Ln+Copy+Square (id 6 =
        # natural_log_exp_and_others in act_info.json) so the table-load
        # insertion pass never has to switch tables mid-kernel
        nc.scalar.add_instruction(mybir.InstLoadActFuncSet(
            name=nc.get_next_instruction_name(), act_func_set_id=6,
            engine=mybir.EngineType.Activation, ins=[], outs=[]))

        # transposed q|k, segments: 0,1 = q head-pairs; 2,3 = k head-pairs
        qkT = persist.tile([128, 4, N], BF16, tag="qkT")
        # transposed normalized attention output, head-pairs, input to proj
        oT = persist.tile([128, 2, N], BF16, tag="oT")

        xt = [persist.tile([128, N], BF16, tag=f"xt{cb}", name=f"xt{cb}")
              for cb in range(CB)]
        wq = [persist.tile([128, WQ], BF16, tag=f"wq{cb}", name=f"wq{cb}")
              for cb in range(CB)]
        wp_t = [persist.tile([128, C], BF16, tag=f"wp{p}", name=f"wp{p}")
                for p in range(2)]

        if not fast_gb:
            gt = persist.tile([128, 512], F32, tag="gt")
            bt = persist.tile([128, 512], F32, tag="bt")
            nc.sync.dma_start(gt[:], gt_e[:])
            nc.sync.dma_start(bt[:], bt_e[:])

        va_pool = ctx.enter_context(tc.tile_pool(name="va", bufs=NB))
        va = [None] * NB

        ptp = ctx.enter_context(tc.tile_pool(name="pt", bufs=32))
        natp = ctx.enter_context(tc.tile_pool(name="nat", bufs=4))
        sqp = ctx.enter_context(tc.tile_pool(name="sq", bufs=6))
        stp = ctx.enter_context(tc.tile_pool(name="st", bufs=32))
        osp = ctx.enter_context(tc.tile_pool(name="os", bufs=6))
        obp = ctx.enter_context(tc.tile_pool(name="ob", bufs=6))
        # PSUM: qp serves qkv blocks + proj halves; sp serves S tiles + o
        qp = ctx.enter_context(tc.tile_pool(name="qp", bufs=2, space="PSUM"))
        sp = ctx.enter_context(tc.tile_pool(name="sp", bufs=2, space="PSUM"))

        # ---- input DMAs: weights + first column wave first ----
        for cb in range(CB):
            nc.sync.dma_start(wq[cb][:], w_e[128 * cb:128 * (cb + 1), :])
            nc.sync.dma_start(xt[cb][:, 0:256],
                              xt_e[128 * cb:128 * (cb + 1), 0:256])
        for cb in range(CB):
            nc.sync.dma_start(xt[cb][:, 256:N],
                              xt_e[128 * cb:128 * (cb + 1), 256:N])
        for p in range(2):
            nc.sync.dma_start(wp_t[p][:], wp_e[128 * p:128 * (p + 1), :])

        # ---- qkv + LN + transposes + v_aug for one 128-row block ----
        def emit_nb(nb):
            n0 = 128 * nb
            qps = qp.tile([128, WQ], F32, tag="qkv", name=f"qkv{nb}")
            for cb in range(CB):
                st = (cb == 0)
                spf = (cb == CB - 1)
                nc.tensor.matmul(qps[:, 0:512], xt[cb][:, n0:n0 + 128],
                                 wq[cb][:, 0:512], start=st, stop=spf)
                nc.tensor.matmul(qps[:, 512:WQ], xt[cb][:, n0:n0 + 128],
                                 wq[cb][:, 512:WQ], start=st, stop=spf)
            nat = natp.tile([128, WQ], BF16, tag="nat", name=f"nat{nb}")
            sq = sqp.tile([128, 512], BF16, tag="sq", name=f"sq{nb}")
            if nb < 4:
                # prologue: ACT is idle until the first exp; DVE is the
                # phase-1 pacer, so do the copy there
                nc.scalar.activation(nat[:], qps[:], func=AF.Copy)
                nc.vector.tensor_tensor(sq[:], nat[:, 0:512], nat[:, 0:512],
                                        op=OP.mult)
            else:
                nc.vector.tensor_copy(nat[:], qps[:])
                nc.vector.tensor_tensor(sq[:], nat[:, 0:512], nat[:, 0:512],
                                        op=OP.mult)

            mean = stp.tile([128, 8], F32, tag="mean", name=f"mean{nb}")
            nc.vector.tensor_scalar(mean[:], qps[:, 512:520], 1.0 / Dh, None,
                                    op0=OP.mult)
            sqs = stp.tile([128, 8], F32, tag="sqs", name=f"sqs{nb}")
            nc.vector.tensor_reduce(sqs[:],
                                    sq[:].rearrange("p (g d) -> p g d", g=8),
                                    axis=mybir.AxisListType.X, op=OP.add)
            msq = stp.tile([128, 8], F32, tag="msq", name=f"msq{nb}")
            nc.vector.tensor_tensor(msq[:], mean[:], mean[:], op=OP.mult)
            rstd = stp.tile([128, 8], F32, tag="rstd", name=f"rstd{nb}")
            nc.vector.scalar_tensor_tensor(rstd[:], sqs[:], 1.0 / Dh, msq[:],
                                           op0=OP.mult, op1=OP.subtract)
            # rstd = (var+eps)^-0.5 = exp(-0.5*ln(var+eps)); Ln/Exp share an
            # ACT table with the softmax Exp, so no act-table reloads
            nc.scalar.activation(rstd[:], rstd[:], func=AF.Ln, bias=eps_t[:])
            nc.scalar.activation(rstd[:], rstd[:], func=AF.Exp, scale=-0.5)

            qk3 = nat[:, 0:512].rearrange("p (g d) -> p g d", g=8)
            nc.vector.tensor_tensor(qk3, qk3, _bc3(mean[:], Dh),
                                    op=OP.subtract)
            nc.vector.tensor_tensor(qk3, qk3, _bc3(rstd[:], Dh), op=OP.mult)
            if not fast_gb:
                nc.vector.tensor_tensor(nat[:, 0:512], nat[:, 0:512], gt[:],
                                        op=OP.mult)
                nc.vector.tensor_tensor(nat[:, 0:512], nat[:, 0:512], bt[:],
                                        op=OP.add)

            nc.sync.dma_start(qkT[:, :, n0:n0 + 128], nat[:, 0:512],
                              transpose=True)

            vat = va_pool.tile([128, HPC, Dh + 1], BF16, tag="vat",
                               name=f"vat{nb}")
            nc.gpsimd.tensor_copy(
                vat[:, :, 0:Dh],
                nat[:, 520:776].rearrange("p (h d) -> p h d", h=HPC))
            nc.gpsimd.tensor_copy(vat[:, :, Dh:Dh + 1],
                                  ones_t[:].to_broadcast([128, HPC, 1]))
            va[nb] = vat

        def width(ib, jb):
            if not causal or jb < 4 * ib:
                return 512
            return 512 - 128 * (jb - 4 * ib)

        # ---- S + exp + mask for one (ib, h, jp); returns the pt tile ----
        def emit_sjp(ib, h, jp):
            p, off = h // 2, 64 * (h % 2)
            i0 = 512 * ib
            jbs = (2 * jp, 2 * jp + 1)
            ws = [width(ib, jb) for jb in jbs]
            # storage: half0 right-aligned to 512, half1 left-aligned at 512
            # -> live cols [512-ws0, 512+ws1) always contiguous
            s_ps = sp.tile([128, 1024], F32, tag="sps",
                           name=f"s{ib}_{h}_{jp}")
            c0s = [512 - ws[0], 512]
            for half, jb in enumerate(jbs):
                w = ws[half]
                nc.tensor.matmul(
                    s_ps[:, c0s[half]:c0s[half] + w],
                    qkT[off:off + Dh, 2 + p, 128 * jb:128 * (jb + 1)],
                    qkT[off:off + Dh, p, i0 + 512 - w:i0 + 512],
                    start=True, stop=True)
            pt = ptp.tile([128, 1024], BF16, tag="pt",
                          name=f"pt{ib}_{h}_{jp}")
            ebias = 0.0 if exp_bias == 0.0 else eb_t[:]
            nc.scalar.activation(pt[:, c0s[0]:512 + ws[1]],
                                 s_ps[:, c0s[0]:512 + ws[1]],
                                 func=AF.Exp, scale=SCALE, bias=ebias)
            for half, jb in enumerate(jbs):
                if causal and jb >= 4 * ib:
                    # triangle lives in the first live 128 cols of this half
                    t = 128 * (jb - 4 * ib)
                    c = (t if half == 0 else 512)
                    nc.gpsimd.affine_select(
                        out=pt[:, c:c + 128], in_=pt[:, c:c + 128],
                        compare_op=OP.is_ge, fill=0.0, base=0,
                        pattern=[[1, 128]], channel_multiplier=-1)
            return pt

        # pt column of block jb for query chunk g (global 128-chunk index)
        def pt_col(ib, jb, half, g):
            bp = g - 4 * ib
            if half == 0:
                return 128 * bp          # right-aligned or full: col = 128*bp
            return 512 + 128 * bp - (512 - width(ib, jb))

        # ---- PV + normalize + O-transpose for all 4 i-chunks of ib ----
        def emit_pv(ib, pts):
            for bp in range(4):
                g = 4 * ib + bp
                jmax = g + 1 if causal else NB
                o_t = sp.tile([128, 1024], F32, tag="sps",
                              name=f"o{ib}_{bp}")
                o_ps = o_t[:, 0:HPC * (Dh + 1)].rearrange(
                    "p (h d) -> p h d", h=HPC)
                for h in range(HPC):
                    for jb in range(jmax):
                        jp, half = jb // 2, jb % 2
                        col = pt_col(ib, jb, half, g)
                        pt = pts[(h, jp)]
                        nc.tensor.matmul(
                            o_ps[:, h, :], pt[:, col:col + 128],
                            va[jb][:, h, :],
                            start=(jb == 0), stop=(jb == jmax - 1))
                rd = stp.tile([128, HPC, 1], F32, tag="rd",
                              name=f"rd{ib}_{bp}")
                nc.vector.reciprocal(rd[:], o_ps[:, :, Dh:Dh + 1])
                osb = osp.tile([128, HPC, Dh], BF16, tag="osb",
                               name=f"osb{ib}_{bp}")
                nc.vector.tensor_tensor(osb[:], o_ps[:, :, 0:Dh],
                                        _bc3(rd[:, :, 0], Dh), op=OP.mult)
                n0 = 128 * g
                nc.sync.dma_start(oT[:, :, n0:n0 + 128], osb[:],
                                  transpose=True)

        # ---- output projection for one 128-row block ----
        def emit_proj(nb):
            n0 = 128 * nb
            ob = obp.tile([128, C], BF16, tag="ob", name=f"ob{nb}")
            for j2 in range(2):
                pp_t = qp.tile([128, WQ], F32, tag="qkv", name=f"pp{nb}_{j2}")
                pp = pp_t[:, 0:512]
                nc.tensor.matmul(pp[:], oT[:, 0, n0:n0 + 128],
                                 wp_t[0][:, 512 * j2:512 * (j2 + 1)],
                                 start=True, stop=False)
                nc.tensor.matmul(pp[:], oT[:, 1, n0:n0 + 128],
                                 wp_t[1][:, 512 * j2:512 * (j2 + 1)],
                                 start=False, stop=True)
                nc.vector.tensor_copy(ob[:, 512 * j2:512 * (j2 + 1)], pp[:])
            nc.sync.dma_start(out_e[n0:n0 + 128, :], ob[:])

        # ---- interleaved emission ----
        def interleave(units, extras):
            if not extras:
                for u in units:
                    u()
                return
            k = len(units) / (len(extras) + 1)
            nxt, ei = k * 2.0, 0
            for i, u in enumerate(units):
                u()
                while ei < len(extras) and i + 1 >= nxt:
                    extras[ei]()
                    ei += 1
                    nxt += k
            while ei < len(extras):
                extras[ei]()
                ei += 1

        for nb in range(4):
            emit_nb(nb)
        proj_sched = {3: [0, 1, 2]}
        for k in range(IB):
            pts = {}
            units = []
            jmax = 4 * (k + 1) if causal else NB
            for h in range(HPC):
                for jp in range(jmax // 2):
                    units.append(lambda ib=k, h=h, jp=jp:
                                 pts.__setitem__((h, jp), emit_sjp(ib, h, jp)))
            extras = []
            if k < IB - 1:
                extras += [lambda nb=nb: emit_nb(nb)
                           for nb in range(4 * (k + 1), 4 * (k + 2))]
            for pib in proj_sched.get(k, []):
                extras += [lambda nb=nb: emit_proj(nb)
                           for nb in range(4 * pib, 4 * pib + 4)]
            interleave(units, extras)
            emit_pv(k, pts)
        for nb in range(4 * (IB - 1), 4 * IB):
            emit_proj(nb)
    return nc


def kernel(x, W_qkv, W_proj, b_proj, ln_g, ln_b, causal, _trace=False):
    global LAST_RESULT
    x = np.asarray(x, dtype=np.float32)
    W_qkv = np.asarray(W_qkv, dtype=np.float32)
    W_proj = np.asarray(W_proj, dtype=np.float32)
    b_proj = np.asarray(b_proj, dtype=np.float32)
    ln_g = np.asarray(ln_g, dtype=np.float32)
    ln_b = np.asarray(ln_b, dtype=np.float32)
    causal = bool(int(np.asarray(causal)))

    fast_gb = bool(np.all(ln_g == 1.0) and np.all(ln_b == 0.0))
    exp_bias = 0.0
    if not fast_gb:
        m = float(SCALE * (8.0 * np.abs(ln_g).max() + 8.0 * np.abs(ln_b).max()) ** 2)
        exp_bias = -max(0.0, m - 8.0)

    key = (causal, fast_gb, exp_bias)
    if key not in _BUILD_CACHE:
        nc = _build(causal, fast_gb, exp_bias)
        nc.finalize()
        _BUILD_CACHE[key] = nc
    nc = _BUILD_CACHE[key]

    xts = [np.ascontiguousarray(x[b].T).astype(BF) for b in range(B)]
    in_maps = []
    for c in range(NCORES):
        b, h0 = c // HPC, Dh * HPC * (c % HPC)   # h0 in channel units
        rq = W_qkv[h0:h0 + 256]
        rk = W_qkv[C + h0:C + h0 + 256]
        rv = W_qkv[2 * C + h0:2 * C + h0 + 256]
        sums = np.concatenate(
            [rq.reshape(4, Dh, C).sum(axis=1), rk.reshape(4, Dh, C).sum(axis=1)])
        w_all = np.concatenate([rq, rk, sums, rv])        # [776, 1024]
        im = {
            "xt": xts[b],
            "w_all": np.ascontiguousarray(w_all.T).astype(BF),
            "wp_t": np.ascontiguousarray(W_proj[:, h0:h0 + 256].T).astype(BF),
        }
        if not fast_gb:
            gseg = np.tile(ln_g, 8)              # q heads x4 then k heads x4
            bseg = np.tile(ln_b, 8)
            im["g_bcast"] = np.broadcast_to(gseg, (128, 512)).copy()
            im["b_bcast"] = np.broadcast_to(bseg, (128, 512)).copy()
        in_maps.append(im)

    res = run_bass_kernel_spmd(nc, in_maps, core_ids=list(range(NCORES)),
                               trace=_trace)
    LAST_RESULT = res

    out = np.empty((B, N, C), dtype=np.float32)
    for b in range(B):
        acc = res.results[4 * b]["out_p"].astype(np.float32)
        for c in range(4 * b + 1, 4 * b + 4):
            acc = acc + res.results[c]["out_p"].astype(np.float32)
        out[b] = acc + b_proj
    return out


# revision 57
# speedup vs baseline: 1.2094x; 1.0047x over previous
"""Trainium2 Bass kernel for BasicSelfAttention (B=2, N=2048, C=1024, H=16, Dh=64).

Sharding: 8 cores = 2 batches x 4 head-groups. Core c handles batch c//4 and
heads [4*(c%4), 4*(c%4)+4).

v2 design (vs. the fp32r baseline):
  - bf16 everywhere on the PE (qkv, scores, PV, proj); fp32 only in PSUM and
    LN statistics. Halves DMA traffic and makes narrow matmul blocks cheap.
  - per-head mean removal via extra weight columns: host appends 8 "row-sum"
    columns to W_qkv so the qkv matmul also produces sum_d(q) per head.
  - rstd = exp(-0.5*ln(var+eps)): Ln/Exp/Copy/Square share one ACT table
    (preloaded once as id 6), so no activation-table reloads ever happen.
  - all transposes via DMA XBAR (dma transpose, s-major row mapping verified
    on HW), freeing the PE and the psum->sbuf copy engines.
  - PV matmul in natural orientation: stationary = P^T block [j,128i],
    moving = v_aug [j,65] -> out [128i, 65] in PSUM at full partition
    utilization (half the PE cost of the [65,w] orientation), and the
    softmax denominator becomes a per-partition scalar: normalize is one
    reciprocal + one broadcast multiply, no DRAM-bounce broadcast.
  - phase-interleaved emission: qkv row-blocks, attention (S/exp/mask),
    PV+normalize, and the output projection are interleaved so PE stays fed
    while ACT grinds through exp.
Host: pre-transposes x and weight slices (bf16), sums the 4 partial
projections per batch, adds b_proj.
"""

import numpy as np
from contextlib import ExitStack

import ml_dtypes
import concourse.bass as bass
import concourse.mybir as mybir
import concourse.tile as tile
from concourse import bacc
from concourse.bass_utils import run_bass_kernel_spmd

B, N, C, H, Dh = 2, 2048, 1024, 16, 64
HPC = 4                      # heads per core
NCORES = 8
SCALE = 8.0 / Dh             # 0.125 (use_mup)
EPS = 1e-5

F32 = mybir.dt.float32
BF16 = mybir.dt.bfloat16
AF = mybir.ActivationFunctionType
OP = mybir.AluOpType
BF = ml_dtypes.bfloat16

NB = N // 128                # 16 row blocks of 128
CB = C // 128                # 8 contraction blocks
IB = N // 512                # 4 query blocks of 512
WQ = 776                     # 256 q | 256 k | 8 head-sums | 256 v

_BUILD_CACHE = {}
LAST_RESULT = None


def _bc3(ap2d, inner):
    """[p, g] AP -> [p, g, inner] with stride-0 inner dim."""
    return bass.AP(tensor=ap2d.tensor, offset=ap2d.offset,
                   ap=list(ap2d.ap) + [[0, inner]])


def _build(causal: bool, fast_gb: bool, exp_bias: float):
    nc = bacc.Bacc("TRN2", target_bir_lowering=False, debug=False,
                   num_devices=NCORES)

    xt_e = nc.dram_tensor("xt", [C, N], BF16, kind="ExternalInput")
    w_e = nc.dram_tensor("w_all", [C, WQ], BF16, kind="ExternalInput")
    wp_e = nc.dram_tensor("wp_t", [HPC * Dh, C], BF16, kind="ExternalInput")
    if not fast_gb:
        gt_e = nc.dram_tensor("g_bcast", [128, 512], F32, kind="ExternalInput")
        bt_e = nc.dram_tensor("b_bcast", [128, 512], F32, kind="ExternalInput")
    out_e = nc.dram_tensor("out_p", [N, C], BF16, kind="ExternalOutput")

    with tile.TileContext(nc) as tc, ExitStack() as ctx:
        persist = ctx.enter_context(tc.tile_pool(name="persist", bufs=1))
        ones_t = persist.tile([128, 1], BF16, tag="ones")
        nc.vector.memset(ones_t[:], 1.0)
        eps_t = persist.tile([128, 1], F32, tag="eps")
        nc.vector.memset(eps_t[:], EPS)
        eb_t = persist.tile([128, 1], F32, tag="ebias")
        nc.vector.memset(eb_t[:], exp_bias)

        # preload the one ACT table that holds Exp+Ln+Copy+Square (id 6 =
        # natural_log_exp_and_others in act_info.json) so the table-load
        # insertion pass never has to switch tables mid-kernel
        nc.scalar.add_instruction(mybir.InstLoadActFuncSet(
            name=nc.get_next_instruction_name(), act_func_set_id=6,
            engine=mybir.EngineType.Activation, ins=[], outs=[]))

        # transposed q|k, segments: 0,1 = q head-pairs; 2,3 = k head-pairs
        qkT = persist.tile([128, 4, N], BF16, tag="qkT")
        # transposed normalized attention output, head-pairs, input to proj
        oT = persist.tile([128, 2, N], BF16, tag="oT")

        xt = [persist.tile([128, N], BF16, tag=f"xt{cb}", name=f"xt{cb}")
              for cb in range(CB)]
        wq = [persist.tile([128, WQ], BF16, tag=f"wq{cb}", name=f"wq{cb}")
              for cb in range(CB)]
        wp_t = [persist.tile([128, C], BF16, tag=f"wp{p}", name=f"wp{p}")
                for p in range(2)]

        if not fast_gb:
            gt = persist.tile([128, 512], F32, tag="gt")
            bt = persist.tile([128, 512], F32, tag="bt")
            nc.sync.dma_start(gt[:], gt_e[:])
            nc.sync.dma_start(bt[:], bt_e[:])

        va_pool = ctx.enter_context(tc.tile_pool(name="va", bufs=NB))
        va = [None] * NB

        ptp = ctx.enter_context(tc.tile_pool(name="pt", bufs=32))
        natp = ctx.enter_context(tc.tile_pool(name="nat", bufs=5))
        sqp = ctx.enter_context(tc.tile_pool(name="sq", bufs=6))
        stp = ctx.enter_context(tc.tile_pool(name="st", bufs=32))
        osp = ctx.enter_context(tc.tile_pool(name="os", bufs=6))
        obp = ctx.enter_context(tc.tile_pool(name="ob", bufs=8))
        # PSUM: qp serves qkv blocks + proj halves; sp serves S tiles + o
        qp = ctx.enter_context(tc.tile_pool(name="qp", bufs=2, space="PSUM"))
        sp = ctx.enter_context(tc.tile_pool(name="sp", bufs=2, space="PSUM"))

        # ---- input DMAs: weights + first column wave first ----
        for cb in range(CB):
            nc.sync.dma_start(wq[cb][:], w_e[128 * cb:128 * (cb + 1), :])
            nc.sync.dma_start(xt[cb][:, 0:256],
                              xt_e[128 * cb:128 * (cb + 1), 0:256])
        for cb in range(CB):
            nc.sync.dma_start(xt[cb][:, 256:N],
                              xt_e[128 * cb:128 * (cb + 1), 256:N])
        for p in range(2):
            nc.sync.dma_start(wp_t[p][:], wp_e[128 * p:128 * (p + 1), :])

        # ---- qkv + LN + transposes + v_aug for one 128-row block ----
        def emit_nb(nb):
            n0 = 128 * nb
            qps = qp.tile([128, WQ], F32, tag="qkv", name=f"qkv{nb}")
            for cb in range(CB):
                st = (cb == 0)
                spf = (cb == CB - 1)
                nc.tensor.matmul(qps[:, 0:512], xt[cb][:, n0:n0 + 128],
                                 wq[cb][:, 0:512], start=st, stop=spf)
                nc.tensor.matmul(qps[:, 512:WQ], xt[cb][:, n0:n0 + 128],
                                 wq[cb][:, 512:WQ], start=st, stop=spf)
            nat = natp.tile([128, WQ], BF16, tag="nat", name=f"nat{nb}")
            sq = sqp.tile([128, 512], BF16, tag="sq", name=f"sq{nb}")
            if nb < 4:
                # prologue: ACT is idle until the first exp; DVE is the
                # phase-1 pacer, so do the copy there
                nc.scalar.activation(nat[:], qps[:], func=AF.Copy)
                nc.vector.tensor_tensor(sq[:], nat[:, 0:512], nat[:, 0:512],
                                        op=OP.mult)
            else:
                nc.vector.tensor_copy(nat[:], qps[:])
                nc.vector.tensor_tensor(sq[:], nat[:, 0:512], nat[:, 0:512],
                                        op=OP.mult)

            mean = stp.tile([128, 8], F32, tag="mean", name=f"mean{nb}")
            nc.vector.tensor_scalar(mean[:], qps[:, 512:520], 1.0 / Dh, None,
                                    op0=OP.mult)
            sqs = stp.tile([128, 8], F32, tag="sqs", name=f"sqs{nb}")
            nc.vector.tensor_reduce(sqs[:],
                                    sq[:].rearrange("p (g d) -> p g d", g=8),
                                    axis=mybir.AxisListType.X, op=OP.add)
            msq = stp.tile([128, 8], F32, tag="msq", name=f"msq{nb}")
            nc.vector.tensor_tensor(msq[:], mean[:], mean[:], op=OP.mult)
            rstd = stp.tile([128, 8], F32, tag="rstd", name=f"rstd{nb}")
            nc.vector.scalar_tensor_tensor(rstd[:], sqs[:], 1.0 / Dh, msq[:],
                                           op0=OP.mult, op1=OP.subtract)
            # rstd = (var+eps)^-0.5 = exp(-0.5*ln(var+eps)); Ln/Exp share an
            # ACT table with the softmax Exp, so no act-table reloads
            nc.scalar.activation(rstd[:], rstd[:], func=AF.Ln, bias=eps_t[:])
            nc.scalar.activation(rstd[:], rstd[:], func=AF.Exp, scale=-0.5)

            qk3 = nat[:, 0:512].rearrange("p (g d) -> p g d", g=8)
            nc.vector.tensor_tensor(qk3, qk3, _bc3(mean[:], Dh),
                                    op=OP.subtract)
            nc.vector.tensor_tensor(qk3, qk3, _bc3(rstd[:], Dh), op=OP.mult)
            if not fast_gb:
                nc.vector.tensor_tensor(nat[:, 0:512], nat[:, 0:512], gt[:],
                                        op=OP.mult)
                nc.vector.tensor_tensor(nat[:, 0:512], nat[:, 0:512], bt[:],
                                        op=OP.add)

            nc.sync.dma_start(qkT[:, :, n0:n0 + 128], nat[:, 0:512],
                              transpose=True)

            vat = va_pool.tile([128, HPC, Dh + 1], BF16, tag="vat",
                               name=f"vat{nb}")
            nc.gpsimd.tensor_copy(
                vat[:, :, 0:Dh],
                nat[:, 520:776].rearrange("p (h d) -> p h d", h=HPC))
            nc.gpsimd.tensor_copy(vat[:, :, Dh:Dh + 1],
                                  ones_t[:].to_broadcast([128, HPC, 1]))
            va[nb] = vat

        def width(ib, jb):
            if not causal or jb < 4 * ib:
                return 512
            return 512 - 128 * (jb - 4 * ib)

        # ---- S + exp + mask for one (ib, h, jp); returns the pt tile ----
        def emit_sjp(ib, h, jp):
            p, off = h // 2, 64 * (h % 2)
            i0 = 512 * ib
            jbs = (2 * jp, 2 * jp + 1)
            ws = [width(ib, jb) for jb in jbs]
            # storage: half0 right-aligned to 512, half1 left-aligned at 512
            # -> live cols [512-ws0, 512+ws1) always contiguous
            s_ps = sp.tile([128, 1024], F32, tag="sps",
                           name=f"s{ib}_{h}_{jp}")
            c0s = [512 - ws[0], 512]
            for half, jb in enumerate(jbs):
                w = ws[half]
                nc.tensor.matmul(
                    s_ps[:, c0s[half]:c0s[half] + w],
                    qkT[off:off + Dh, 2 + p, 128 * jb:128 * (jb + 1)],
                    qkT[off:off + Dh, p, i0 + 512 - w:i0 + 512],
                    start=True, stop=True)
            pt = ptp.tile([128, 1024], BF16, tag="pt",
                          name=f"pt{ib}_{h}_{jp}")
            ebias = 0.0 if exp_bias == 0.0 else eb_t[:]
            nc.scalar.activation(pt[:, c0s[0]:512 + ws[1]],
                                 s_ps[:, c0s[0]:512 + ws[1]],
                                 func=AF.Exp, scale=SCALE, bias=ebias)
            for half, jb in enumerate(jbs):
                if causal and jb >= 4 * ib:
                    # triangle lives in the first live 128 cols of this half
                    t = 128 * (jb - 4 * ib)
                    c = (t if half == 0 else 512)
                    nc.gpsimd.affine_select(
                        out=pt[:, c:c + 128], in_=pt[:, c:c + 128],
                        compare_op=OP.is_ge, fill=0.0, base=0,
                        pattern=[[1, 128]], channel_multiplier=-1)
            return pt

        # pt column of block jb for query chunk g (global 128-chunk index)
        def pt_col(ib, jb, half, g):
            bp = g - 4 * ib
            if half == 0:
                return 128 * bp          # right-aligned or full: col = 128*bp
            return 512 + 128 * bp - (512 - width(ib, jb))

        # ---- PV + normalize + O-transpose for all 4 i-chunks of ib ----
        def emit_pv(ib, pts):
            for bp in range(4):
                g = 4 * ib + bp
                jmax = g + 1 if causal else NB
                o_t = sp.tile([128, 1024], F32, tag="sps",
                              name=f"o{ib}_{bp}")
                o_ps = o_t[:, 0:HPC * (Dh + 1)].rearrange(
                    "p (h d) -> p h d", h=HPC)
                for h in range(HPC):
                    for jb in range(jmax):
                        jp, half = jb // 2, jb % 2
                        col = pt_col(ib, jb, half, g)
                        pt = pts[(h, jp)]
                        nc.tensor.matmul(
                            o_ps[:, h, :], pt[:, col:col + 128],
                            va[jb][:, h, :],
                            start=(jb == 0), stop=(jb == jmax - 1))
                rd = stp.tile([128, HPC, 1], F32, tag="rd",
                              name=f"rd{ib}_{bp}")
                nc.vector.reciprocal(rd[:], o_ps[:, :, Dh:Dh + 1])
                osb = osp.tile([128, HPC, Dh], BF16, tag="osb",
                               name=f"osb{ib}_{bp}")
                nc.vector.tensor_tensor(osb[:], o_ps[:, :, 0:Dh],
                                        _bc3(rd[:, :, 0], Dh), op=OP.mult)
                n0 = 128 * g
                nc.sync.dma_start(oT[:, :, n0:n0 + 128], osb[:],
                                  transpose=True)

        # ---- output projection for one 128-row block ----
        def emit_proj(nb):
            n0 = 128 * nb
            ob = obp.tile([128, C], BF16, tag="ob", name=f"ob{nb}")
            for j2 in range(2):
                pp_t = qp.tile([128, WQ], F32, tag="qkv", name=f"pp{nb}_{j2}")
                pp = pp_t[:, 0:512]
                nc.tensor.matmul(pp[:], oT[:, 0, n0:n0 + 128],
                                 wp_t[0][:, 512 * j2:512 * (j2 + 1)],
                                 start=True, stop=False)
                nc.tensor.matmul(pp[:], oT[:, 1, n0:n0 + 128],
                                 wp_t[1][:, 512 * j2:512 * (j2 + 1)],
                                 start=False, stop=True)
                nc.vector.tensor_copy(ob[:, 512 * j2:512 * (j2 + 1)], pp[:])
            nc.sync.dma_start(out_e[n0:n0 + 128, :], ob[:])

        # ---- interleaved emission ----
        def interleave(units, extras):
            if not extras:
                for u in units:
                    u()
                return
            k = len(units) / (len(extras) + 1)
            nxt, ei = k * 2.0, 0
            for i, u in enumerate(units):
                u()
                while ei < len(extras) and i + 1 >= nxt:
                    extras[ei]()
                    ei += 1
                    nxt += k
            while ei < len(extras):
                extras[ei]()
                ei += 1

        for nb in range(4):
            emit_nb(nb)
        proj_sched = {3: [0, 1, 2]}
        for k in range(IB):
            pts = {}
            units = []
            jmax = 4 * (k + 1) if causal else NB
            for h in range(HPC):
                for jp in range(jmax // 2):
                    units.append(lambda ib=k, h=h, jp=jp:
                                 pts.__setitem__((h, jp), emit_sjp(ib, h, jp)))
            extras = []
            if k < IB - 1:
                extras += [lambda nb=nb: emit_nb(nb)
                           for nb in range(4 * (k + 1), 4 * (k + 2))]
            for pib in proj_sched.get(k, []):
                extras += [lambda nb=nb: emit_proj(nb)
                           for nb in range(4 * pib, 4 * pib + 4)]
            interleave(units, extras)
            emit_pv(k, pts)
        for nb in range(4 * (IB - 1), 4 * IB):
            emit_proj(nb)
    return nc


def kernel(x, W_qkv, W_proj, b_proj, ln_g, ln_b, causal, _trace=False):
    global LAST_RESULT
    x = np.asarray(x, dtype=np.float32)
    W_qkv = np.asarray(W_qkv, dtype=np.float32)
    W_proj = np.asarray(W_proj, dtype=np.float32)
    b_proj = np.asarray(b_proj, dtype=np.float32)
    ln_g = np.asarray(ln_g, dtype=np.float32)
    ln_b = np.asarray(ln_b, dtype=np.float32)
    causal = bool(int(np.asarray(causal)))

    fast_gb = bool(np.all(ln_g == 1.0) and np.all(ln_b == 0.0))
    exp_bias = 0.0
    if not fast_gb:
        m = float(SCALE * (8.0 * np.abs(ln_g).max() + 8.0 * np.abs(ln_b).max()) ** 2)
        exp_bias = -max(0.0, m - 8.0)

    key = (causal, fast_gb, exp_bias)
    if key not in _BUILD_CACHE:
        nc = _build(causal, fast_gb, exp_bias)
        nc.finalize()
        _BUILD_CACHE[key] = nc
    nc = _BUILD_CACHE[key]

    xts = [np.ascontiguousarray(x[b].T).astype(BF) for b in range(B)]
    in_maps = []
    for c in range(NCORES):
        b, h0 = c // HPC, Dh * HPC * (c % HPC)   # h0 in channel units
        rq = W_qkv[h0:h0 + 256]
        rk = W_qkv[C + h0:C + h0 + 256]
        rv = W_qkv[2 * C + h0:2 * C + h0 + 256]
        sums = np.concatenate(
            [rq.reshape(4, Dh, C).sum(axis=1), rk.reshape(4, Dh, C).sum(axis=1)])
        w_all = np.concatenate([rq, rk, sums, rv])        # [776, 1024]
        im = {
            "xt": xts[b],
            "w_all": np.ascontiguousarray(w_all.T).astype(BF),
            "wp_t": np.ascontiguousarray(W_proj[:, h0:h0 + 256].T).astype(BF),
        }
        if not fast_gb:
            gseg = np.tile(ln_g, 8)              # q heads x4 then k heads x4
            bseg = np.tile(ln_b, 8)
            im["g_bcast"] = np.broadcast_to(gseg, (128, 512)).copy()
            im["b_bcast"] = np.broadcast_to(bseg, (128, 512)).copy()
        in_maps.append(im)

    res = run_bass_kernel_spmd(nc, in_maps, core_ids=list(range(NCORES)),
                               trace=_trace)
    LAST_RESULT = res

    out = np.empty((B, N, C), dtype=np.float32)
    for b in range(B):
        acc = res.results[4 * b]["out_p"].astype(np.float32)
        for c in range(4 * b + 1, 4 * b + 4):
            acc = acc + res.results[c]["out_p"].astype(np.float32)
        out[b] = acc + b_proj
    return out


# revision 59
# speedup vs baseline: 1.2560x; 1.0385x over previous
"""Trainium2 Bass kernel for BasicSelfAttention (B=2, N=2048, C=1024, H=16, Dh=64).

Sharding: 8 cores = 2 batches x 4 head-groups. Core c handles batch c//4 and
heads [4*(c%4), 4*(c%4)+4).

v2 design (vs. the fp32r baseline):
  - bf16 everywhere on the PE (qkv, scores, PV, proj); fp32 only in PSUM and
    LN statistics. Halves DMA traffic and makes narrow matmul blocks cheap.
  - per-head mean removal via extra weight columns: host appends 8 "row-sum"
    columns to W_qkv so the qkv matmul also produces sum_d(q) per head.
  - rstd = exp(-0.5*ln(var+eps)): Ln/Exp/Copy/Square share one ACT table
    (preloaded once as id 6), so no activation-table reloads ever happen.
  - all transposes via DMA XBAR (dma transpose, s-major row mapping verified
    on HW), freeing the PE and the psum->sbuf copy engines.
  - PV matmul in natural orientation: stationary = P^T block [j,128i],
    moving = v_aug [j,65] -> out [128i, 65] in PSUM at full partition
    utilization (half the PE cost of the [65,w] orientation), and the
    softmax denominator becomes a per-partition scalar: normalize is one
    reciprocal + one broadcast multiply, no DRAM-bounce broadcast.
  - phase-interleaved emission: qkv row-blocks, attention (S/exp/mask),
    PV+normalize, and the output projection are interleaved so PE stays fed
    while ACT grinds through exp.
Host: pre-transposes x and weight slices (bf16), sums the 4 partial
projections per batch, adds b_proj.
"""

import numpy as np
from contextlib import ExitStack

import ml_dtypes
import concourse.bass as bass
import concourse.mybir as mybir
import concourse.tile as tile
from concourse import bacc
from concourse.bass_utils import run_bass_kernel_spmd

B, N, C, H, Dh = 2, 2048, 1024, 16, 64
HPC = 4                      # heads per core
NCORES = 8
SCALE = 8.0 / Dh             # 0.125 (use_mup)
EPS = 1e-5

F32 = mybir.dt.float32
BF16 = mybir.dt.bfloat16
AF = mybir.ActivationFunctionType
OP = mybir.AluOpType
BF = ml_dtypes.bfloat16

NB = N // 128                # 16 row blocks of 128
CB = C // 128                # 8 contraction blocks
IB = N // 512                # 4 query blocks of 512
WQ = 776                     # 256 q | 256 k | 8 head-sums | 256 v

_BUILD_CACHE = {}
LAST_RESULT = None


def _bc3(ap2d, inner):
    """[p, g] AP -> [p, g, inner] with stride-0 inner dim."""
    return bass.AP(tensor=ap2d.tensor, offset=ap2d.offset,
                   ap=list(ap2d.ap) + [[0, inner]])


def _build(causal: bool, fast_gb: bool, exp_bias: float):
    nc = bacc.Bacc("TRN2", target_bir_lowering=False, debug=False,
                   num_devices=NCORES)

    xt_e = nc.dram_tensor("xt", [C, N], BF16, kind="ExternalInput")
    w_e = nc.dram_tensor("w_all", [C, WQ], BF16, kind="ExternalInput")
    wp_e = nc.dram_tensor("wp_t", [HPC * Dh, C], BF16, kind="ExternalInput")
    if not fast_gb:
        gt_e = nc.dram_tensor("g_bcast", [128, 512], F32, kind="ExternalInput")
        bt_e = nc.dram_tensor("b_bcast", [128, 512], F32, kind="ExternalInput")
    out_e = nc.dram_tensor("out_p", [N, C], BF16, kind="ExternalOutput")

    with tile.TileContext(nc) as tc, ExitStack() as ctx:
        persist = ctx.enter_context(tc.tile_pool(name="persist", bufs=1))
        ones_t = persist.tile([128, 1], BF16, tag="ones")
        nc.vector.memset(ones_t[:], 1.0)
        eps_t = persist.tile([128, 1], F32, tag="eps")
        nc.vector.memset(eps_t[:], EPS)
        eb_t = persist.tile([128, 1], F32, tag="ebias")
        nc.vector.memset(eb_t[:], exp_bias)

        # preload the one ACT table that holds Exp+Ln+Copy+Square (id 6 =
        # natural_log_exp_and_others in act_info.json) so the table-load
        # insertion pass never has to switch tables mid-kernel
        nc.scalar.add_instruction(mybir.InstLoadActFuncSet(
            name=nc.get_next_instruction_name(), act_func_set_id=6,
            engine=mybir.EngineType.Activation, ins=[], outs=[]))

        # transposed q|k, segments: 0,1 = q head-pairs; 2,3 = k head-pairs
        qkT = persist.tile([128, 4, N], BF16, tag="qkT")
        # transposed normalized attention output, head-pairs, input to proj
        oT = persist.tile([128, 2, N], BF16, tag="oT")

        xt = [persist.tile([128, N], BF16, tag=f"xt{cb}", name=f"xt{cb}")
              for cb in range(CB)]
        wq = [persist.tile([128, WQ], BF16, tag=f"wq{cb}", name=f"wq{cb}")
              for cb in range(CB)]
        wp_t = [persist.tile([128, C], BF16, tag=f"wp{p}", name=f"wp{p}")
                for p in range(2)]

        if not fast_gb:
            gt = persist.tile([128, 512], F32, tag="gt")
            bt = persist.tile([128, 512], F32, tag="bt")
            nc.sync.dma_start(gt[:], gt_e[:])
            nc.sync.dma_start(bt[:], bt_e[:])

        va_pool = ctx.enter_context(tc.tile_pool(name="va", bufs=NB))
        va = [None] * NB

        ptp = ctx.enter_context(tc.tile_pool(name="pt", bufs=32))
        natp = ctx.enter_context(tc.tile_pool(name="nat", bufs=5))
        sqp = ctx.enter_context(tc.tile_pool(name="sq", bufs=6))
        stp = ctx.enter_context(tc.tile_pool(name="st", bufs=32))
        osp = ctx.enter_context(tc.tile_pool(name="os", bufs=6))
        obp = ctx.enter_context(tc.tile_pool(name="ob", bufs=8))
        # PSUM: qp serves qkv blocks + proj halves; sp serves S tiles + o
        qp = ctx.enter_context(tc.tile_pool(name="qp", bufs=2, space="PSUM"))
        sp = ctx.enter_context(tc.tile_pool(name="sp", bufs=2, space="PSUM"))

        # ---- input DMAs: weights + first column wave first ----
        for cb in range(CB):
            nc.sync.dma_start(wq[cb][:], w_e[128 * cb:128 * (cb + 1), :])
            nc.sync.dma_start(xt[cb][:, 0:512],
                              xt_e[128 * cb:128 * (cb + 1), 0:512])
        for cb in range(CB):
            nc.sync.dma_start(xt[cb][:, 512:N],
                              xt_e[128 * cb:128 * (cb + 1), 512:N])
        for p in range(2):
            nc.sync.dma_start(wp_t[p][:], wp_e[128 * p:128 * (p + 1), :])

        # ---- qkv + LN + transposes + v_aug for one 128-row block ----
        def emit_nb(nb):
            n0 = 128 * nb
            qps = qp.tile([128, WQ], F32, tag="qkv", name=f"qkv{nb}")
            for cb in range(CB):
                st = (cb == 0)
                spf = (cb == CB - 1)
                nc.tensor.matmul(qps[:, 0:512], xt[cb][:, n0:n0 + 128],
                                 wq[cb][:, 0:512], start=st, stop=spf)
                nc.tensor.matmul(qps[:, 512:WQ], xt[cb][:, n0:n0 + 128],
                                 wq[cb][:, 512:WQ], start=st, stop=spf)
            nat = natp.tile([128, WQ], BF16, tag="nat", name=f"nat{nb}")
            sq = sqp.tile([128, 512], BF16, tag="sq", name=f"sq{nb}")
            if nb < 4:
                # prologue: ACT is idle until the first exp; DVE is the
                # phase-1 pacer, so do the copy there
                nc.scalar.activation(nat[:], qps[:], func=AF.Copy)
                nc.vector.tensor_tensor(sq[:], nat[:, 0:512], nat[:, 0:512],
                                        op=OP.mult)
            else:
                nc.vector.tensor_copy(nat[:], qps[:])
                nc.vector.tensor_tensor(sq[:], nat[:, 0:512], nat[:, 0:512],
                                        op=OP.mult)

            mean = stp.tile([128, 8], F32, tag="mean", name=f"mean{nb}")
            nc.vector.tensor_scalar(mean[:], qps[:, 512:520], 1.0 / Dh, None,
                                    op0=OP.mult)
            sqs = stp.tile([128, 8], F32, tag="sqs", name=f"sqs{nb}")
            nc.vector.tensor_reduce(sqs[:],
                                    sq[:].rearrange("p (g d) -> p g d", g=8),
                                    axis=mybir.AxisListType.X, op=OP.add)
            msq = stp.tile([128, 8], F32, tag="msq", name=f"msq{nb}")
            nc.vector.tensor_tensor(msq[:], mean[:], mean[:], op=OP.mult)
            rstd = stp.tile([128, 8], F32, tag="rstd", name=f"rstd{nb}")
            nc.vector.scalar_tensor_tensor(rstd[:], sqs[:], 1.0 / Dh, msq[:],
                                           op0=OP.mult, op1=OP.subtract)
            # rstd = (var+eps)^-0.5 = exp(-0.5*ln(var+eps)); Ln/Exp share an
            # ACT table with the softmax Exp, so no act-table reloads
            nc.scalar.activation(rstd[:], rstd[:], func=AF.Ln, bias=eps_t[:])
            nc.scalar.activation(rstd[:], rstd[:], func=AF.Exp, scale=-0.5)

            qk3 = nat[:, 0:512].rearrange("p (g d) -> p g d", g=8)
            nc.vector.tensor_tensor(qk3, qk3, _bc3(mean[:], Dh),
                                    op=OP.subtract)
            nc.vector.tensor_tensor(qk3, qk3, _bc3(rstd[:], Dh), op=OP.mult)
            if not fast_gb:
                nc.vector.tensor_tensor(nat[:, 0:512], nat[:, 0:512], gt[:],
                                        op=OP.mult)
                nc.vector.tensor_tensor(nat[:, 0:512], nat[:, 0:512], bt[:],
                                        op=OP.add)

            nc.sync.dma_start(qkT[:, :, n0:n0 + 128], nat[:, 0:512],
                              transpose=True)

            vat = va_pool.tile([128, HPC, Dh + 1], BF16, tag="vat",
                               name=f"vat{nb}")
            nc.gpsimd.tensor_copy(
                vat[:, :, 0:Dh],
                nat[:, 520:776].rearrange("p (h d) -> p h d", h=HPC))
            nc.gpsimd.tensor_copy(vat[:, :, Dh:Dh + 1],
                                  ones_t[:].to_broadcast([128, HPC, 1]))
            va[nb] = vat

        def width(ib, jb):
            if not causal or jb < 4 * ib:
                return 512
            return 512 - 128 * (jb - 4 * ib)

        # ---- S + exp + mask for one (ib, h, jp); returns the pt tile ----
        def emit_sjp(ib, h, jp):
            p, off = h // 2, 64 * (h % 2)
            i0 = 512 * ib
            jbs = (2 * jp, 2 * jp + 1)
            ws = [width(ib, jb) for jb in jbs]
            # storage: half0 right-aligned to 512, half1 left-aligned at 512
            # -> live cols [512-ws0, 512+ws1) always contiguous
            s_ps = sp.tile([128, 1024], F32, tag="sps",
                           name=f"s{ib}_{h}_{jp}")
            c0s = [512 - ws[0], 512]
            for half, jb in enumerate(jbs):
                w = ws[half]
                nc.tensor.matmul(
                    s_ps[:, c0s[half]:c0s[half] + w],
                    qkT[off:off + Dh, 2 + p, 128 * jb:128 * (jb + 1)],
                    qkT[off:off + Dh, p, i0 + 512 - w:i0 + 512],
                    start=True, stop=True)
            pt = ptp.tile([128, 1024], BF16, tag="pt",
                          name=f"pt{ib}_{h}_{jp}")
            ebias = 0.0 if exp_bias == 0.0 else eb_t[:]
            nc.scalar.activation(pt[:, c0s[0]:512 + ws[1]],
                                 s_ps[:, c0s[0]:512 + ws[1]],
                                 func=AF.Exp, scale=SCALE, bias=ebias)
            for half, jb in enumerate(jbs):
                if causal and jb >= 4 * ib:
                    # triangle lives in the first live 128 cols of this half
                    t = 128 * (jb - 4 * ib)
                    c = (t if half == 0 else 512)
                    nc.gpsimd.affine_select(
                        out=pt[:, c:c + 128], in_=pt[:, c:c + 128],
                        compare_op=OP.is_ge, fill=0.0, base=0,
                        pattern=[[1, 128]], channel_multiplier=-1)
            return pt

        # pt column of block jb for query chunk g (global 128-chunk index)
        def pt_col(ib, jb, half, g):
            bp = g - 4 * ib
            if half == 0:
                return 128 * bp          # right-aligned or full: col = 128*bp
            return 512 + 128 * bp - (512 - width(ib, jb))

        # ---- PV + normalize + O-transpose for all 4 i-chunks of ib ----
        def emit_pv(ib, pts):
            for bp in range(4):
                g = 4 * ib + bp
                jmax = g + 1 if causal else NB
                o_t = sp.tile([128, 1024], F32, tag="sps",
                              name=f"o{ib}_{bp}")
                o_ps = o_t[:, 0:HPC * (Dh + 1)].rearrange(
                    "p (h d) -> p h d", h=HPC)
                for h in range(HPC):
                    for jb in range(jmax):
                        jp, half = jb // 2, jb % 2
                        col = pt_col(ib, jb, half, g)
                        pt = pts[(h, jp)]
                        nc.tensor.matmul(
                            o_ps[:, h, :], pt[:, col:col + 128],
                            va[jb][:, h, :],
                            start=(jb == 0), stop=(jb == jmax - 1))
                rd = stp.tile([128, HPC, 1], F32, tag="rd",
                              name=f"rd{ib}_{bp}")
                nc.vector.reciprocal(rd[:], o_ps[:, :, Dh:Dh + 1])
                osb = osp.tile([128, HPC, Dh], BF16, tag="osb",
                               name=f"osb{ib}_{bp}")
                nc.vector.tensor_tensor(osb[:], o_ps[:, :, 0:Dh],
                                        _bc3(rd[:, :, 0], Dh), op=OP.mult)
                n0 = 128 * g
                nc.sync.dma_start(oT[:, :, n0:n0 + 128], osb[:],
                                  transpose=True)

        # ---- output projection for one 128-row block ----
        def emit_proj(nb):
            n0 = 128 * nb
            ob = obp.tile([128, C], BF16, tag="ob", name=f"ob{nb}")
            for j2 in range(2):
                pp_t = qp.tile([128, WQ], F32, tag="qkv", name=f"pp{nb}_{j2}")
                pp = pp_t[:, 0:512]
                nc.tensor.matmul(pp[:], oT[:, 0, n0:n0 + 128],
                                 wp_t[0][:, 512 * j2:512 * (j2 + 1)],
                                 start=True, stop=False)
                nc.tensor.matmul(pp[:], oT[:, 1, n0:n0 + 128],
                                 wp_t[1][:, 512 * j2:512 * (j2 + 1)],
                                 start=False, stop=True)
                if nb >= 12 and j2 == 1:
                    nc.scalar.activation(ob[:, 512:1024], pp[:], func=AF.Copy)
                else:
                    nc.vector.tensor_copy(ob[:, 512 * j2:512 * (j2 + 1)],
                                          pp[:])
            nc.sync.dma_start(out_e[n0:n0 + 128, :], ob[:])

        # ---- interleaved emission ----
        def interleave(units, extras):
            if not extras:
                for u in units:
                    u()
                return
            k = len(units) / (len(extras) + 1)
            nxt, ei = k * 2.0, 0
            for i, u in enumerate(units):
                u()
                while ei < len(extras) and i + 1 >= nxt:
                    extras[ei]()
                    ei += 1
                    nxt += k
            while ei < len(extras):
                extras[ei]()
                ei += 1

        for nb in range(4):
            emit_nb(nb)
        proj_sched = {3: [0, 1, 2]}
        for k in range(IB):
            pts = {}
            units = []
            jmax = 4 * (k + 1) if causal else NB
            for h in range(HPC):
                for jp in range(jmax // 2):
                    units.append(lambda ib=k, h=h, jp=jp:
                                 pts.__setitem__((h, jp), emit_sjp(ib, h, jp)))
            extras = []
            if k < IB - 1:
                extras += [lambda nb=nb: emit_nb(nb)
                           for nb in range(4 * (k + 1), 4 * (k + 2))]
            for pib in proj_sched.get(k, []):
                extras += [lambda nb=nb: emit_proj(nb)
                           for nb in range(4 * pib, 4 * pib + 4)]
            interleave(units, extras)
            emit_pv(k, pts)
        for nb in range(4 * (IB - 1), 4 * IB):
            emit_proj(nb)
    return nc


def kernel(x, W_qkv, W_proj, b_proj, ln_g, ln_b, causal, _trace=False):
    global LAST_RESULT
    x = np.asarray(x, dtype=np.float32)
    W_qkv = np.asarray(W_qkv, dtype=np.float32)
    W_proj = np.asarray(W_proj, dtype=np.float32)
    b_proj = np.asarray(b_proj, dtype=np.float32)
    ln_g = np.asarray(ln_g, dtype=np.float32)
    ln_b = np.asarray(ln_b, dtype=np.float32)
    causal = bool(int(np.asarray(causal)))

    fast_gb = bool(np.all(ln_g == 1.0) and np.all(ln_b == 0.0))
    exp_bias = 0.0
    if not fast_gb:
        m = float(SCALE * (8.0 * np.abs(ln_g).max() + 8.0 * np.abs(ln_b).max()) ** 2)
        exp_bias = -max(0.0, m - 8.0)

    key = (causal, fast_gb, exp_bias)
    if key not in _BUILD_CACHE:
        nc = _build(causal, fast_gb, exp_bias)
        nc.finalize()
        _BUILD_CACHE[key] = nc
    nc = _BUILD_CACHE[key]

    xts = [np.ascontiguousarray(x[b].T).astype(BF) for b in range(B)]
    in_maps = []
    for c in range(NCORES):
        b, h0 = c // HPC, Dh * HPC * (c % HPC)   # h0 in channel units
        rq = W_qkv[h0:h0 + 256]
        rk = W_qkv[C + h0:C + h0 + 256]
        rv = W_qkv[2 * C + h0:2 * C + h0 + 256]
        sums = np.concatenate(
            [rq.reshape(4, Dh, C).sum(axis=1), rk.reshape(4, Dh, C).sum(axis=1)])
        w_all = np.concatenate([rq, rk, sums, rv])        # [776, 1024]
        im = {
            "xt": xts[b],
            "w_all": np.ascontiguousarray(w_all.T).astype(BF),
            "wp_t": np.ascontiguousarray(W_proj[:, h0:h0 + 256].T).astype(BF),
        }
        if not fast_gb:
            gseg = np.tile(ln_g, 8)              # q heads x4 then k heads x4
            bseg = np.tile(ln_b, 8)
            im["g_bcast"] = np.broadcast_to(gseg, (128, 512)).copy()
            im["b_bcast"] = np.broadcast_to(bseg, (128, 512)).copy()
        in_maps.append(im)

    res = run_bass_kernel_spmd(nc, in_maps, core_ids=list(range(NCORES)),
                               trace=_trace)
    LAST_RESULT = res

    out = np.empty((B, N, C), dtype=np.float32)
    for b in range(B):
        acc = res.results[4 * b]["out_p"].astype(np.float32)
        for c in range(4 * b + 1, 4 * b + 4):
            acc = acc + res.results[c]["out_p"].astype(np.float32)
        out[b] = acc + b_proj
    return out
